# revision 3
# baseline (speedup 1.0000x reference)
"""Cross-attention Trainium2 kernel (8-core data-parallel over batch).

Per-core computation (one batch element per NeuronCore):
  q = x @ Wq; k = ctx @ Wk; v = ctx @ Wv
  attn = softmax((q k^T) / sqrt(dh)); out = attn @ v; y = out @ Wo + bo

v2 structure (vs the v1 baseline):
  - weights arrive fp32 over the two HWDGE rings (sync/scalar) and are cast
    to bf16 on ACT/DVE, so the serial SWDGE (gpsimd cast-load) queue only
    carries the per-group x tiles -> the PE starts ~10us earlier.
  - xT comes from the DMA XBAR transpose (dma_start(transpose=True)) instead
    of PE transposes + DVE copies: out[p, c, t] = in[t, c*128+p] gives the
    [qd, tok] chunk layout directly.
  - scores per head pair are emitted back-to-back as 64x128 row tiles
    (partition bases 0/64) into one 2-bank PSUM tile; the PE runs both
    concurrently (measured pair span ~320ns vs 432 serial), and a single
    ACT exp sweeps the 2-bank tile.
  - softmax denominators: col-tiled (128x64) pair matmuls with a shared
    [77->128, 64] all-ones bf16 stationary write sum_ctx(exp_h) broadcast
    across the 64 out partitions of each head; same tile mode as the
    attention-output pair matmuls (no extra PE mode switch).
  - attn-output pairs unchanged (col tiles 0/64), evicted with *1/rowsum.
  - y stores ride the sync HWDGE ring; everything on-chip stays bf16 for
    matmul operands with fp32 PSUM accumulation.
"""

import numpy as np

import concourse.bass as bass
import concourse.tile as tile
from concourse import bacc, mybir
from concourse.bass_utils import run_bass_kernel_spmd
from concourse.masks import make_identity

F32 = mybir.dt.float32
BF16 = mybir.dt.bfloat16

B, N, M = 8, 4096, 77
QD, CD, H, DH = 512, 768, 8, 64
INNER = H * DH  # 512
P = 128
S = 512  # token group size
NQC = QD // P  # 4 qd chunks
NCC = CD // P  # 6 cd chunks
NIC = INNER // P  # 4 inner chunks
NTS = S // P  # 4 token sub-tiles per group
SCALE = DH ** -0.5
MP = 128  # context length padded to full partition width (zeros are inert)


def build_kernel(groups: int = N // S):
    nc = bacc.Bacc(None, target_bir_lowering=False, debug=False)

    x_d = nc.dram_tensor("x", [N, QD], F32, kind="ExternalInput")
    ctx_d = nc.dram_tensor("context", [M, CD], F32, kind="ExternalInput")
    wq_d = nc.dram_tensor("Wq", [QD, INNER], F32, kind="ExternalInput")
    wk_d = nc.dram_tensor("Wk", [CD, INNER], F32, kind="ExternalInput")
    wv_d = nc.dram_tensor("Wv", [CD, INNER], F32, kind="ExternalInput")
    wo_d = nc.dram_tensor("Wo", [INNER, QD], F32, kind="ExternalInput")
    bo_d = nc.dram_tensor("bo", [QD], F32, kind="ExternalInput")
    y_d = nc.dram_tensor("y", [N, QD], F32, kind="ExternalOutput")

    from contextlib import ExitStack

    with tile.TileContext(nc) as tc, ExitStack() as st:
        consts = st.enter_context(tc.tile_pool(name="consts", bufs=1))
        kvp = st.enter_context(tc.tile_pool(name="kv", bufs=1))
        xin = st.enter_context(tc.tile_pool(name="xin", bufs=3))
        xtp = st.enter_context(tc.tile_pool(name="xt", bufs=2))
        qtp = st.enter_context(tc.tile_pool(name="qt", bufs=2))
        expp = st.enter_context(tc.tile_pool(name="expp", bufs=2))
        rcp = st.enter_context(tc.tile_pool(name="rcp", bufs=2))
        outp = st.enter_context(tc.tile_pool(name="outp", bufs=2))
        yp = st.enter_context(tc.tile_pool(name="yp", bufs=2))

        # PSUM: 8 banks. qf 2x[128,512] (2) + s 2x[128,2,512] (4) + r 2x[128,512] (2)
        ps_qf = st.enter_context(tc.tile_pool(name="ps_qf", bufs=2, space="PSUM"))
        ps_s = st.enter_context(tc.tile_pool(name="ps_s", bufs=2, space="PSUM"))
        ps_r = st.enter_context(tc.tile_pool(name="ps_r", bufs=2, space="PSUM"))

        # ---- tiny constants (engines only, no DMA) ------------------------------
        identity = consts.tile([P, P], BF16)
        make_identity(nc, identity)

        # all-ones [77, 64] stationary (zero-padded to 128 rows) for the
        # col-tiled rowsum matmuls
        ones64 = consts.tile([MP, DH], BF16)
        nc.vector.memset(ones64, 0.0)
        nc.vector.memset(ones64[:M, :], 1.0)

        # ---- fp32 weight loads over the two HWDGE rings -------------------------
        ctx32 = consts.tile([MP, CD], F32)
        nc.vector.memset(ctx32, 0.0)
        nc.sync.dma_start(out=ctx32[:M, :], in_=ctx_d[:, :])
        wk32 = consts.tile([P, NCC, INNER], F32)
        nc.scalar.dma_start(out=wk32, in_=wk_d.ap().rearrange("(c p) n -> p c n", p=P))
        wq32 = consts.tile([P, NQC, INNER], F32)
        nc.sync.dma_start(out=wq32, in_=wq_d.ap().rearrange("(c p) n -> p c n", p=P))
        wv32 = consts.tile([P, NCC, INNER], F32)
        nc.scalar.dma_start(out=wv32, in_=wv_d.ap().rearrange("(c p) n -> p c n", p=P))

        # ---- SWDGE: x cast-loads only (plus tiny bo broadcast) ------------------
        def load_x(g):
            x_g = xin.tile([P, NTS, QD], BF16)
            nc.gpsimd.dma_start(
                out=x_g,
                in_=x_d[g * S : (g + 1) * S, :].rearrange("(t p) q -> p t q", p=P),
            )
            return x_g

        x_pre = [load_x(0), load_x(1)]

        bo_bc = consts.tile([P, QD], F32)
        bo_ap = bo_d.ap()
        nc.gpsimd.dma_start(
            out=bo_bc, in_=bass.AP(bo_ap.tensor, bo_ap.offset, [[0, P], [1, QD]])
        )

        # ---- casts to bf16 (ACT for the early-critical ones, DVE for the rest) --
        ctx_sb = consts.tile([MP, CD], BF16)
        nc.scalar.copy(out=ctx_sb, in_=ctx32)
        wk_sb = consts.tile([P, NCC, INNER], BF16)
        nc.scalar.copy(out=wk_sb, in_=wk32)
        wq_sb = consts.tile([P, NQC, INNER], BF16)
        nc.vector.tensor_copy(out=wq_sb, in_=wq32)
        wv_sb = consts.tile([P, NCC, INNER], BF16)
        nc.vector.tensor_copy(out=wv_sb, in_=wv32)

        # Wo rides the sync ring after the XBAR transposes of group 0 (it is
        # first needed one pipeline stage later).
        wo32 = consts.tile([P, NIC, QD], F32)
        wo_sb = consts.tile([P, NIC, QD], BF16)

        # ---- context projections (tiny) -----------------------------------------
        ctxT = kvp.tile([P, NCC, MP], BF16)
        for cc in range(NCC):
            pt = ps_r.tile([P, MP], BF16, tag="ps_r")
            nc.tensor.transpose(pt, ctx_sb[:, cc * P : (cc + 1) * P], identity)
            nc.vector.tensor_copy(out=ctxT[:, cc, :], in_=pt)

        kT = kvp.tile([P, NIC, MP], BF16)
        for ic in range(NIC):
            pk = ps_qf.tile([P, MP], F32, tag="ps_qf")
            for cc in range(NCC):
                nc.tensor.matmul(
                    pk,
                    wk_sb[:, cc, ic * P : (ic + 1) * P],
                    ctxT[:, cc, :],
                    start=(cc == 0),
                    stop=(cc == NCC - 1),
                )
            nc.vector.tensor_copy(out=kT[:, ic, :], in_=pk)

        v_sb = kvp.tile([MP, INNER], BF16)
        pv = ps_s.tile([MP, INNER], F32, tag="ps_s")
        for cc in range(NCC):
            nc.tensor.matmul(
                pv,
                ctxT[:, cc, :],
                wv_sb[:, cc, :],
                start=(cc == 0),
                stop=(cc == NCC - 1),
            )
        nc.vector.tensor_copy(out=v_sb, in_=pv)

        # ---- XBAR transpose: x natural [tok, qd] -> xT [qd, tok] ----------------
        def xbar_x(g, x_g):
            xT = xtp.tile([P, NQC, S], BF16)
            for ts in range(NTS):
                nc.sync.dma_start(
                    out=xT[:, :, ts * P : (ts + 1) * P],
                    in_=x_g[:, ts, :],
                    transpose=True,
                )
            return xT

        xT_pre = [xbar_x(0, x_pre[0])]

        # deferred Wo load + cast
        nc.sync.dma_start(out=wo32, in_=wo_d.ap().rearrange("(c p) n -> p c n", p=P))
        nc.vector.tensor_copy(out=wo_sb, in_=wo32)

        # ---- main loop over token groups ----------------------------------------
        # Software-pipelined emission: group g's rowsums / attention-output /
        # final projection are emitted one iteration later, after group g+1's
        # front block, so the scheduler can fill PE stalls (exp latency)
        # with back-phase matmuls.

        def emit_front(g):
            x_g = x_pre[g]
            if g + 2 < groups:
                x_pre.append(load_x(g + 2))
            if g + 1 < groups:
                xT_pre.append(xbar_x(g + 1, x_pre[g + 1]))

            xT = xT_pre[g]
            # qT[inner, tok]
            qT = qtp.tile([P, NIC, S], BF16)
            for ic in range(NIC):
                pq = ps_qf.tile([P, S], F32, tag="ps_qf")
                for c in range(NQC):
                    nc.tensor.matmul(
                        pq,
                        wq_sb[:, c, ic * P : (ic + 1) * P],
                        xT[:, c, :],
                        start=(c == 0),
                        stop=(c == NQC - 1),
                    )
                nc.scalar.copy(out=qT[:, ic, :], in_=pq)

            # scores -> exp per head pair: the pair runs as concurrent 64x128
            # row tiles (partition bases 0/64) into one 2-bank psum tile, then
            # one ACT exp sweeps both banks.
            exp_g = expp.tile([MP, H, S], BF16)
            for pp in range(H // 2):
                sp = ps_s.tile([MP, 2, S], F32, tag="ps_s")
                nc.tensor.matmul(
                    sp[:, 0, :],
                    kT[0:DH, pp, :],
                    qT[0:DH, pp, :],
                    start=True,
                    stop=True,
                )
                nc.tensor.matmul(
                    sp[:, 1, :],
                    kT[DH : 2 * DH, pp, :],
                    qT[DH : 2 * DH, pp, :],
                    start=True,
                    stop=True,
                )
                nc.scalar.activation(
                    out=exp_g[:, 2 * pp : 2 * pp + 2, :],
                    in_=sp,
                    func=mybir.ActivationFunctionType.Exp,
                    scale=SCALE,
                )
            return exp_g

        def emit_back(g, exp_g):
            # rowsums, broadcast across each head's 64 out partitions by the
            # col-tiled ones matmuls; then reciprocal per pair
            rec_g = rcp.tile([P, H // 2, S], F32)
            for pp in range(H // 2):
                pr = ps_r.tile([P, S], F32, tag="ps_r")
                nc.tensor.matmul(
                    pr[0:DH, :],
                    ones64,
                    exp_g[:, 2 * pp, :],
                    start=True,
                    stop=True,
                    tile_position=(0, 0),
                )
                nc.tensor.matmul(
                    pr[DH : 2 * DH, :],
                    ones64,
                    exp_g[:, 2 * pp + 1, :],
                    start=True,
                    stop=True,
                    tile_position=(0, DH),
                )
                nc.vector.reciprocal_approx_fast(out=rec_g[:, pp, :], in_=pr)

            # outT (unnormalized) * (1/r); pair-packed into one bank
            outT = outp.tile([P, NIC, S], BF16)
            for pp in range(H // 2):
                po = ps_r.tile([P, S], F32, tag="ps_r")
                for side in range(2):
                    h = 2 * pp + side
                    nc.tensor.matmul(
                        po[side * DH : (side + 1) * DH, :],
                        v_sb[:, h * DH : (h + 1) * DH],
                        exp_g[:, h, :],
                        start=True,
                        stop=True,
                        tile_position=(0, side * DH),
                    )
                nc.vector.tensor_mul(
                    out=outT[:, pp, :], in0=po, in1=rec_g[:, pp, :]
                )

            # final projection + bias
            tok = slice(g * S, (g + 1) * S)
            y_g = yp.tile([P, NTS, QD], F32)
            for ts in range(NTS):
                pf = ps_qf.tile([P, QD], F32, tag="ps_qf")
                for ic in range(NIC):
                    nc.tensor.matmul(
                        pf,
                        outT[:, ic, ts * P : (ts + 1) * P],
                        wo_sb[:, ic, :],
                        start=(ic == 0),
                        stop=(ic == NIC - 1),
                    )
                nc.vector.tensor_add(out=y_g[:, ts, :], in0=pf, in1=bo_bc)

            nc.sync.dma_start(
                out=y_d[tok, :].rearrange("(t p) q -> p t q", p=P), in_=y_g
            )

        pending = None
        for g in range(groups):
            exp_g = emit_front(g)
            if pending is not None:
                emit_back(pending[0], pending[1])
            pending = (g, exp_g)
        emit_back(pending[0], pending[1])

    nc.compile()
    return nc


_CACHE = {}


def _get_nc():
    if "nc" not in _CACHE:
        _CACHE["nc"] = build_kernel()
    return _CACHE["nc"]


def run(inputs, trace=False, **kw):
    nc = _get_nc()
    in_maps = []
    for i in range(B):
        m = {
            "x": np.asarray(inputs["x"][i], dtype=np.float32),
            "context": np.asarray(inputs["context"][i], dtype=np.float32),
            "Wq": np.asarray(inputs["Wq"], dtype=np.float32),
            "Wk": np.asarray(inputs["Wk"], dtype=np.float32),
            "Wv": np.asarray(inputs["Wv"], dtype=np.float32),
            "Wo": np.asarray(inputs["Wo"], dtype=np.float32),
            "bo": np.asarray(inputs["bo"], dtype=np.float32),
        }
        in_maps.append(m)
    res = run_bass_kernel_spmd(nc, in_maps, list(range(B)), trace=trace, **kw)
    out = np.stack([res.results[i]["y"] for i in range(B)], axis=0)
    return out, res


def kernel(**inputs):
    out, _ = run(inputs)
    return out


# revision 8
# speedup vs baseline: 1.0057x; 1.0057x over previous
"""Cross-attention Trainium2 kernel (8-core data-parallel over batch).

Per-core computation (one batch element per NeuronCore):
  q = x @ Wq; k = ctx @ Wk; v = ctx @ Wv
  attn = softmax((q k^T) / sqrt(dh)); out = attn @ v; y = out @ Wo + bo

v2 structure (vs the v1 baseline):
  - weights arrive fp32 over the two HWDGE rings (sync/scalar) and are cast
    to bf16 on ACT/DVE, so the serial SWDGE (gpsimd cast-load) queue only
    carries the per-group x tiles -> the PE starts ~10us earlier.
  - xT via PE transposes + DVE copies (the DMA XBAR transpose path measured
    slower end-to-end: Tile serializes dma_start_transpose against all other
    in-flight DMAs as a HW-deadlock guard, which convoys the x/weight loads).
  - scores per head pair are emitted back-to-back as 64x128 row tiles
    (partition bases 0/64) into one 2-bank PSUM tile; the PE runs both
    concurrently (measured pair span ~320ns vs 432 serial), and a single
    ACT exp sweeps the 2-bank tile.
  - softmax denominators: col-tiled (128x64) pair matmuls with a shared
    [77->128, 64] all-ones bf16 stationary write sum_ctx(exp_h) broadcast
    across the 64 out partitions of each head; same tile mode as the
    attention-output pair matmuls (no extra PE mode switch).
  - attn-output pairs unchanged (col tiles 0/64), evicted with *1/rowsum.
  - y stores ride the sync HWDGE ring; everything on-chip stays bf16 for
    matmul operands with fp32 PSUM accumulation.
"""

import numpy as np

import concourse.bass as bass
import concourse.tile as tile
from concourse import bacc, mybir
from concourse.bass_utils import run_bass_kernel_spmd
from concourse.masks import make_identity

F32 = mybir.dt.float32
BF16 = mybir.dt.bfloat16

B, N, M = 8, 4096, 77
QD, CD, H, DH = 512, 768, 8, 64
INNER = H * DH  # 512
P = 128
S = 512  # token group size
NQC = QD // P  # 4 qd chunks
NCC = CD // P  # 6 cd chunks
NIC = INNER // P  # 4 inner chunks
NTS = S // P  # 4 token sub-tiles per group
SCALE = DH ** -0.5
MP = 128  # context length padded to full partition width (zeros are inert)


def build_kernel(groups: int = N // S):
    nc = bacc.Bacc(None, target_bir_lowering=False, debug=False)

    x_d = nc.dram_tensor("x", [N, QD], F32, kind="ExternalInput")
    ctx_d = nc.dram_tensor("context", [M, CD], F32, kind="ExternalInput")
    wq_d = nc.dram_tensor("Wq", [QD, INNER], F32, kind="ExternalInput")
    wk_d = nc.dram_tensor("Wk", [CD, INNER], F32, kind="ExternalInput")
    wv_d = nc.dram_tensor("Wv", [CD, INNER], F32, kind="ExternalInput")
    wo_d = nc.dram_tensor("Wo", [INNER, QD], F32, kind="ExternalInput")
    bo_d = nc.dram_tensor("bo", [QD], F32, kind="ExternalInput")
    y_d = nc.dram_tensor("y", [N, QD], F32, kind="ExternalOutput")

    from contextlib import ExitStack

    with tile.TileContext(nc) as tc, ExitStack() as st:
        consts = st.enter_context(tc.tile_pool(name="consts", bufs=1))
        kvp = st.enter_context(tc.tile_pool(name="kv", bufs=1))
        xin = st.enter_context(tc.tile_pool(name="xin", bufs=3))
        xtp = st.enter_context(tc.tile_pool(name="xt", bufs=2))
        qtp = st.enter_context(tc.tile_pool(name="qt", bufs=2))
        expp = st.enter_context(tc.tile_pool(name="expp", bufs=2))
        rcp = st.enter_context(tc.tile_pool(name="rcp", bufs=2))
        outp = st.enter_context(tc.tile_pool(name="outp", bufs=2))
        yp = st.enter_context(tc.tile_pool(name="yp", bufs=2))

        # PSUM: 8 banks. qf 2x[128,512] (2) + s 2x[128,2,512] (4) + r 2x[128,512] (2)
        ps_qf = st.enter_context(tc.tile_pool(name="ps_qf", bufs=2, space="PSUM"))
        ps_s = st.enter_context(tc.tile_pool(name="ps_s", bufs=2, space="PSUM"))
        ps_r = st.enter_context(tc.tile_pool(name="ps_r", bufs=2, space="PSUM"))

        # ---- tiny constants (engines only, no DMA) ------------------------------
        identity = consts.tile([P, P], BF16)
        make_identity(nc, identity)

        # all-ones [77, 64] stationary (zero-padded to 128 rows) for the
        # col-tiled rowsum matmuls
        ones64 = consts.tile([MP, DH], BF16)
        nc.vector.memset(ones64, 0.0)
        nc.vector.memset(ones64[:M, :], 1.0)

        # ---- fp32 weight loads over the two HWDGE rings -------------------------
        # sync ring: ctx -> wk -> wq (the early-critical chain); scalar ring:
        # wv + wo, issued before any ACT compute so they never stall the ACT
        # queue behind a data dependency.
        ctx32 = consts.tile([MP, CD], F32)
        nc.vector.memset(ctx32, 0.0)
        nc.sync.dma_start(out=ctx32[:M, :], in_=ctx_d[:, :])
        wv32 = consts.tile([P, NCC, INNER], F32)
        nc.scalar.dma_start(out=wv32, in_=wv_d.ap().rearrange("(c p) n -> p c n", p=P))
        wo32 = consts.tile([P, NIC, QD], F32)
        nc.scalar.dma_start(out=wo32, in_=wo_d.ap().rearrange("(c p) n -> p c n", p=P))
        wk32 = consts.tile([P, NCC, INNER], F32)
        nc.sync.dma_start(out=wk32, in_=wk_d.ap().rearrange("(c p) n -> p c n", p=P))
        wq32 = consts.tile([P, NQC, INNER], F32)
        nc.sync.dma_start(out=wq32, in_=wq_d.ap().rearrange("(c p) n -> p c n", p=P))

        # ---- SWDGE: x cast-loads only (plus tiny bo broadcast) ------------------
        def load_x(g):
            x_g = xin.tile([P, NTS, QD], BF16)
            nc.gpsimd.dma_start(
                out=x_g,
                in_=x_d[g * S : (g + 1) * S, :].rearrange("(t p) q -> p t q", p=P),
            )
            return x_g

        x_pre = [load_x(0), load_x(1)]

        bo_bc = consts.tile([P, QD], F32)
        bo_ap = bo_d.ap()
        nc.gpsimd.dma_start(
            out=bo_bc, in_=bass.AP(bo_ap.tensor, bo_ap.offset, [[0, P], [1, QD]])
        )

        # ---- casts to bf16 (ACT for the early-critical ones, DVE for the rest) --
        ctx_sb = consts.tile([MP, CD], BF16)
        nc.scalar.copy(out=ctx_sb, in_=ctx32)
        wk_sb = consts.tile([P, NCC, INNER], BF16)
        nc.scalar.copy(out=wk_sb, in_=wk32)
        wq_sb = consts.tile([P, NQC, INNER], BF16)
        nc.vector.tensor_copy(out=wq_sb, in_=wq32)
        wv_sb = consts.tile([P, NCC, INNER], BF16)
        nc.vector.tensor_copy(out=wv_sb, in_=wv32)
        wo_sb = consts.tile([P, NIC, QD], BF16)
        nc.vector.tensor_copy(out=wo_sb, in_=wo32)

        # ---- context projections (tiny) -----------------------------------------
        ctxT = kvp.tile([P, NCC, MP], BF16)
        for cc in range(NCC):
            pt = ps_r.tile([P, MP], BF16, tag="ps_r")
            nc.tensor.transpose(pt, ctx_sb[:, cc * P : (cc + 1) * P], identity)
            nc.vector.tensor_copy(out=ctxT[:, cc, :], in_=pt)

        kT = kvp.tile([P, NIC, MP], BF16)
        for ic in range(NIC):
            pk = ps_qf.tile([P, MP], F32, tag="ps_qf")
            for cc in range(NCC):
                nc.tensor.matmul(
                    pk,
                    wk_sb[:, cc, ic * P : (ic + 1) * P],
                    ctxT[:, cc, :],
                    start=(cc == 0),
                    stop=(cc == NCC - 1),
                )
            nc.vector.tensor_copy(out=kT[:, ic, :], in_=pk)

        v_sb = kvp.tile([MP, INNER], BF16)
        pv = ps_s.tile([MP, INNER], F32, tag="ps_s")
        for cc in range(NCC):
            nc.tensor.matmul(
                pv,
                ctxT[:, cc, :],
                wv_sb[:, cc, :],
                start=(cc == 0),
                stop=(cc == NCC - 1),
            )
        nc.vector.tensor_copy(out=v_sb, in_=pv)

        # ---- main loop over token groups ----------------------------------------
        # Software-pipelined emission: group g's rowsums / attention-output /
        # final projection are emitted one iteration later, after group g+1's
        # front block, so the scheduler can fill PE stalls (exp latency)
        # with back-phase matmuls.

        def emit_front(g):
            x_g = x_pre[g]
            if g + 2 < groups:
                x_pre.append(load_x(g + 2))

            # transpose x tiles: xT[p, c, t*128+j] = x[t*128+..., c*128+p];
            # 4 PE transposes land in one psum bank, one DVE copy per chunk
            xT = xtp.tile([P, NQC, S], BF16)
            for c in range(NQC):
                pt = ps_r.tile([P, S], BF16, tag="ps_r")
                for ts in range(NTS):
                    nc.tensor.transpose(
                        pt[:, ts * P : (ts + 1) * P],
                        x_g[:, ts, c * P : (c + 1) * P],
                        identity,
                    )
                nc.vector.tensor_copy(out=xT[:, c, :], in_=pt)

            # qT[inner, tok]
            qT = qtp.tile([P, NIC, S], BF16)
            for ic in range(NIC):
                pq = ps_qf.tile([P, S], F32, tag="ps_qf")
                for c in range(NQC):
                    nc.tensor.matmul(
                        pq,
                        wq_sb[:, c, ic * P : (ic + 1) * P],
                        xT[:, c, :],
                        start=(c == 0),
                        stop=(c == NQC - 1),
                    )
                nc.scalar.copy(out=qT[:, ic, :], in_=pq)

            # scores -> exp per head pair: the pair runs as concurrent 64x128
            # row tiles (partition bases 0/64) into one 2-bank psum tile, then
            # one ACT exp sweeps both banks.
            exp_g = expp.tile([MP, H, S], BF16)
            for pp in range(H // 2):
                sp = ps_s.tile([MP, 2, S], F32, tag="ps_s")
                nc.tensor.matmul(
                    sp[:, 0, :],
                    kT[0:DH, pp, :],
                    qT[0:DH, pp, :],
                    start=True,
                    stop=True,
                )
                nc.tensor.matmul(
                    sp[:, 1, :],
                    kT[DH : 2 * DH, pp, :],
                    qT[DH : 2 * DH, pp, :],
                    start=True,
                    stop=True,
                )
                nc.scalar.activation(
                    out=exp_g[:, 2 * pp : 2 * pp + 2, :],
                    in_=sp,
                    func=mybir.ActivationFunctionType.Exp,
                    scale=SCALE,
                )
            return exp_g

        def emit_back(g, exp_g):
            # rowsums, broadcast across each head's 64 out partitions by the
            # col-tiled ones matmuls; then reciprocal per pair
            rec_g = rcp.tile([P, H // 2, S], F32)
            for pp in range(H // 2):
                pr = ps_r.tile([P, S], F32, tag="ps_r")
                nc.tensor.matmul(
                    pr[0:DH, :],
                    ones64,
                    exp_g[:, 2 * pp, :],
                    start=True,
                    stop=True,
                    tile_position=(0, 0),
                )
                nc.tensor.matmul(
                    pr[DH : 2 * DH, :],
                    ones64,
                    exp_g[:, 2 * pp + 1, :],
                    start=True,
                    stop=True,
                    tile_position=(0, DH),
                )
                nc.vector.reciprocal_approx_fast(out=rec_g[:, pp, :], in_=pr)

            # outT (unnormalized) * (1/r); pair-packed into one bank
            outT = outp.tile([P, NIC, S], BF16)
            for pp in range(H // 2):
                po = ps_r.tile([P, S], F32, tag="ps_r")
                for side in range(2):
                    h = 2 * pp + side
                    nc.tensor.matmul(
                        po[side * DH : (side + 1) * DH, :],
                        v_sb[:, h * DH : (h + 1) * DH],
                        exp_g[:, h, :],
                        start=True,
                        stop=True,
                        tile_position=(0, side * DH),
                    )
                nc.vector.tensor_mul(
                    out=outT[:, pp, :], in0=po, in1=rec_g[:, pp, :]
                )

            # final projection + bias
            tok = slice(g * S, (g + 1) * S)
            y_g = yp.tile([P, NTS, QD], F32)
            for ts in range(NTS):
                pf = ps_qf.tile([P, QD], F32, tag="ps_qf")
                for ic in range(NIC):
                    nc.tensor.matmul(
                        pf,
                        outT[:, ic, ts * P : (ts + 1) * P],
                        wo_sb[:, ic, :],
                        start=(ic == 0),
                        stop=(ic == NIC - 1),
                    )
                nc.vector.tensor_add(out=y_g[:, ts, :], in0=pf, in1=bo_bc)

            nc.sync.dma_start(
                out=y_d[tok, :].rearrange("(t p) q -> p t q", p=P), in_=y_g
            )

        pending = None
        for g in range(groups):
            exp_g = emit_front(g)
            if pending is not None:
                emit_back(pending[0], pending[1])
            pending = (g, exp_g)
        emit_back(pending[0], pending[1])

    nc.compile()
    return nc


_CACHE = {}


def _get_nc():
    if "nc" not in _CACHE:
        _CACHE["nc"] = build_kernel()
    return _CACHE["nc"]


def run(inputs, trace=False, **kw):
    nc = _get_nc()
    in_maps = []
    for i in range(B):
        m = {
            "x": np.asarray(inputs["x"][i], dtype=np.float32),
            "context": np.asarray(inputs["context"][i], dtype=np.float32),
            "Wq": np.asarray(inputs["Wq"], dtype=np.float32),
            "Wk": np.asarray(inputs["Wk"], dtype=np.float32),
            "Wv": np.asarray(inputs["Wv"], dtype=np.float32),
            "Wo": np.asarray(inputs["Wo"], dtype=np.float32),
            "bo": np.asarray(inputs["bo"], dtype=np.float32),
        }
        in_maps.append(m)
    res = run_bass_kernel_spmd(nc, in_maps, list(range(B)), trace=trace, **kw)
    out = np.stack([res.results[i]["y"] for i in range(B)], axis=0)
    return out, res


def kernel(**inputs):
    out, _ = run(inputs)
    return out


# revision 9
# speedup vs baseline: 1.0819x; 1.0757x over previous
"""Cross-attention Trainium2 kernel (8-core data-parallel over batch).

Per-core computation (one batch element per NeuronCore):
  q = x @ Wq; k = ctx @ Wk; v = ctx @ Wv
  attn = softmax((q k^T) / sqrt(dh)); out = attn @ v; y = out @ Wo + bo

v2 structure (vs the v1 baseline):
  - weights arrive fp32 over the two HWDGE rings (sync/scalar) and are cast
    to bf16 on ACT/DVE, so the serial SWDGE (gpsimd cast-load) queue only
    carries the per-group x tiles -> the PE starts ~10us earlier.
  - xT via PE transposes + DVE copies (the DMA XBAR transpose path measured
    slower end-to-end: Tile serializes dma_start_transpose against all other
    in-flight DMAs as a HW-deadlock guard, which convoys the x/weight loads).
  - scores per head pair are emitted back-to-back as 64x128 row tiles
    (partition bases 0/64) into one 2-bank PSUM tile; the PE runs both
    concurrently (measured pair span ~320ns vs 432 serial), and a single
    ACT exp sweeps the 2-bank tile.
  - softmax denominators: col-tiled (128x64) pair matmuls with a shared
    [77->128, 64] all-ones bf16 stationary write sum_ctx(exp_h) broadcast
    across the 64 out partitions of each head; same tile mode as the
    attention-output pair matmuls (no extra PE mode switch).
  - attn-output pairs unchanged (col tiles 0/64), evicted with *1/rowsum.
  - y stores ride the sync HWDGE ring; everything on-chip stays bf16 for
    matmul operands with fp32 PSUM accumulation.
"""

import numpy as np

import concourse.bass as bass
import concourse.tile as tile
from concourse import bacc, mybir
from concourse.bass_utils import run_bass_kernel_spmd
from concourse.masks import make_identity

F32 = mybir.dt.float32
BF16 = mybir.dt.bfloat16

B, N, M = 8, 4096, 77
QD, CD, H, DH = 512, 768, 8, 64
INNER = H * DH  # 512
P = 128
S = 512  # token group size
NQC = QD // P  # 4 qd chunks
NCC = CD // P  # 6 cd chunks
NIC = INNER // P  # 4 inner chunks
NTS = S // P  # 4 token sub-tiles per group
SCALE = DH ** -0.5
MP = 128  # context length padded to full partition width (zeros are inert)


def build_kernel(groups: int = N // S):
    nc = bacc.Bacc(None, target_bir_lowering=False, debug=False)

    x_d = nc.dram_tensor("x", [N, QD], F32, kind="ExternalInput")
    ctx_d = nc.dram_tensor("context", [M, CD], F32, kind="ExternalInput")
    wq_d = nc.dram_tensor("Wq", [QD, INNER], F32, kind="ExternalInput")
    wk_d = nc.dram_tensor("Wk", [CD, INNER], F32, kind="ExternalInput")
    wv_d = nc.dram_tensor("Wv", [CD, INNER], F32, kind="ExternalInput")
    wo_d = nc.dram_tensor("Wo", [INNER, QD], F32, kind="ExternalInput")
    bo_d = nc.dram_tensor("bo", [QD], F32, kind="ExternalInput")
    y_d = nc.dram_tensor("y", [N, QD], F32, kind="ExternalOutput")

    from contextlib import ExitStack

    with tile.TileContext(nc) as tc, ExitStack() as st:
        consts = st.enter_context(tc.tile_pool(name="consts", bufs=1))
        kvp = st.enter_context(tc.tile_pool(name="kv", bufs=1))
        xin = st.enter_context(tc.tile_pool(name="xin", bufs=3))
        xtp = st.enter_context(tc.tile_pool(name="xt", bufs=2))
        qtp = st.enter_context(tc.tile_pool(name="qt", bufs=2))
        expp = st.enter_context(tc.tile_pool(name="expp", bufs=2))
        rcp = st.enter_context(tc.tile_pool(name="rcp", bufs=2))
        outp = st.enter_context(tc.tile_pool(name="outp", bufs=2))
        yp = st.enter_context(tc.tile_pool(name="yp", bufs=2))

        # PSUM: 8 banks. qf 2x[128,512] (2) + s 2x[128,2,512] (4) + r 2x[128,512] (2)
        ps_qf = st.enter_context(tc.tile_pool(name="ps_qf", bufs=2, space="PSUM"))
        ps_s = st.enter_context(tc.tile_pool(name="ps_s", bufs=2, space="PSUM"))
        ps_r = st.enter_context(tc.tile_pool(name="ps_r", bufs=2, space="PSUM"))

        # ---- tiny constants (engines only, no DMA) ------------------------------
        identity = consts.tile([P, P], BF16)
        make_identity(nc, identity)

        # all-ones [77, 64] stationary (zero-padded to 128 rows) for the
        # col-tiled rowsum matmuls
        ones64 = consts.tile([MP, DH], BF16)
        nc.vector.memset(ones64, 0.0)
        nc.vector.memset(ones64[:M, :], 1.0)

        # ---- loads: one serial SWDGE (gpsimd cast-load) queue, in need order ----
        # A single queue keeps DMA completion order == consumption order; the
        # HWDGE-parallel variant measured worse (SDMA round-robins the rings,
        # so the early-critical small loads finish last).
        def load_x(g):
            x_g = xin.tile([P, NTS, QD], BF16)
            nc.gpsimd.dma_start(
                out=x_g,
                in_=x_d[g * S : (g + 1) * S, :].rearrange("(t p) q -> p t q", p=P),
            )
            return x_g

        ctx_sb = kvp.tile([MP, CD], BF16)
        nc.vector.memset(ctx_sb, 0.0)
        nc.gpsimd.dma_start(out=ctx_sb[:M, :], in_=ctx_d[:, :])

        wk_sb = consts.tile([P, NCC, INNER], BF16)
        nc.gpsimd.dma_start(
            out=wk_sb, in_=wk_d.ap().rearrange("(c p) n -> p c n", p=P)
        )

        x_pre = [load_x(0)]

        wq_sb = consts.tile([P, NQC, INNER], BF16)
        nc.gpsimd.dma_start(
            out=wq_sb, in_=wq_d.ap().rearrange("(c p) n -> p c n", p=P)
        )

        wv_sb = consts.tile([P, NCC, INNER], BF16)
        nc.gpsimd.dma_start(
            out=wv_sb, in_=wv_d.ap().rearrange("(c p) n -> p c n", p=P)
        )

        x_pre.append(load_x(1))

        wo_sb = consts.tile([P, NIC, QD], BF16)
        nc.gpsimd.dma_start(
            out=wo_sb, in_=wo_d.ap().rearrange("(c p) n -> p c n", p=P)
        )

        bo_bc = consts.tile([P, QD], F32)
        bo_ap = bo_d.ap()
        nc.gpsimd.dma_start(
            out=bo_bc, in_=bass.AP(bo_ap.tensor, bo_ap.offset, [[0, P], [1, QD]])
        )

        # ---- context projections (tiny) -----------------------------------------
        ctxT = kvp.tile([P, NCC, MP], BF16)
        for cc in range(NCC):
            pt = ps_r.tile([P, MP], BF16, tag="ps_r")
            nc.tensor.transpose(pt, ctx_sb[:, cc * P : (cc + 1) * P], identity)
            nc.vector.tensor_copy(out=ctxT[:, cc, :], in_=pt)

        kT = kvp.tile([P, NIC, MP], BF16)
        for ic in range(NIC):
            pk = ps_qf.tile([P, MP], F32, tag="ps_qf")
            for cc in range(NCC):
                nc.tensor.matmul(
                    pk,
                    wk_sb[:, cc, ic * P : (ic + 1) * P],
                    ctxT[:, cc, :],
                    start=(cc == 0),
                    stop=(cc == NCC - 1),
                )
            nc.vector.tensor_copy(out=kT[:, ic, :], in_=pk)

        v_sb = kvp.tile([MP, INNER], BF16)
        pv = ps_s.tile([MP, INNER], F32, tag="ps_s")
        for cc in range(NCC):
            nc.tensor.matmul(
                pv,
                ctxT[:, cc, :],
                wv_sb[:, cc, :],
                start=(cc == 0),
                stop=(cc == NCC - 1),
            )
        nc.vector.tensor_copy(out=v_sb, in_=pv)

        # ---- main loop over token groups ----------------------------------------
        # Software-pipelined emission: group g's rowsums / attention-output /
        # final projection are emitted one iteration later, after group g+1's
        # front block, so the scheduler can fill PE stalls (exp latency)
        # with back-phase matmuls.

        def emit_front(g):
            x_g = x_pre[g]
            if g + 2 < groups:
                x_pre.append(load_x(g + 2))

            # transpose x tiles: xT[p, c, t*128+j] = x[t*128+..., c*128+p];
            # 4 PE transposes land in one psum bank, one DVE copy per chunk
            xT = xtp.tile([P, NQC, S], BF16)
            for c in range(NQC):
                pt = ps_r.tile([P, S], BF16, tag="ps_r")
                for ts in range(NTS):
                    nc.tensor.transpose(
                        pt[:, ts * P : (ts + 1) * P],
                        x_g[:, ts, c * P : (c + 1) * P],
                        identity,
                    )
                nc.vector.tensor_copy(out=xT[:, c, :], in_=pt)

            # qT[inner, tok]
            qT = qtp.tile([P, NIC, S], BF16)
            for ic in range(NIC):
                pq = ps_qf.tile([P, S], F32, tag="ps_qf")
                for c in range(NQC):
                    nc.tensor.matmul(
                        pq,
                        wq_sb[:, c, ic * P : (ic + 1) * P],
                        xT[:, c, :],
                        start=(c == 0),
                        stop=(c == NQC - 1),
                    )
                nc.scalar.copy(out=qT[:, ic, :], in_=pq)

            # scores -> exp per head pair: the pair runs as concurrent 64x128
            # row tiles (partition bases 0/64) into one 2-bank psum tile, then
            # one ACT exp sweeps both banks.
            exp_g = expp.tile([MP, H, S], BF16)
            for pp in range(H // 2):
                sp = ps_s.tile([MP, 2, S], F32, tag="ps_s")
                nc.tensor.matmul(
                    sp[:, 0, :],
                    kT[0:DH, pp, :],
                    qT[0:DH, pp, :],
                    start=True,
                    stop=True,
                )
                nc.tensor.matmul(
                    sp[:, 1, :],
                    kT[DH : 2 * DH, pp, :],
                    qT[DH : 2 * DH, pp, :],
                    start=True,
                    stop=True,
                )
                nc.scalar.activation(
                    out=exp_g[:, 2 * pp : 2 * pp + 2, :],
                    in_=sp,
                    func=mybir.ActivationFunctionType.Exp,
                    scale=SCALE,
                )
            return exp_g

        def emit_back(g, exp_g):
            # rowsums, broadcast across each head's 64 out partitions by the
            # col-tiled ones matmuls; then reciprocal per pair
            rec_g = rcp.tile([P, H // 2, S], F32)
            for pp in range(H // 2):
                pr = ps_r.tile([P, S], F32, tag="ps_r")
                nc.tensor.matmul(
                    pr[0:DH, :],
                    ones64,
                    exp_g[:, 2 * pp, :],
                    start=True,
                    stop=True,
                    tile_position=(0, 0),
                )
                nc.tensor.matmul(
                    pr[DH : 2 * DH, :],
                    ones64,
                    exp_g[:, 2 * pp + 1, :],
                    start=True,
                    stop=True,
                    tile_position=(0, DH),
                )
                nc.vector.reciprocal_approx_fast(out=rec_g[:, pp, :], in_=pr)

            # outT (unnormalized) * (1/r); pair-packed into one bank
            outT = outp.tile([P, NIC, S], BF16)
            for pp in range(H // 2):
                po = ps_r.tile([P, S], F32, tag="ps_r")
                for side in range(2):
                    h = 2 * pp + side
                    nc.tensor.matmul(
                        po[side * DH : (side + 1) * DH, :],
                        v_sb[:, h * DH : (h + 1) * DH],
                        exp_g[:, h, :],
                        start=True,
                        stop=True,
                        tile_position=(0, side * DH),
                    )
                nc.vector.tensor_mul(
                    out=outT[:, pp, :], in0=po, in1=rec_g[:, pp, :]
                )

            # final projection + bias
            tok = slice(g * S, (g + 1) * S)
            y_g = yp.tile([P, NTS, QD], F32)
            for ts in range(NTS):
                pf = ps_qf.tile([P, QD], F32, tag="ps_qf")
                for ic in range(NIC):
                    nc.tensor.matmul(
                        pf,
                        outT[:, ic, ts * P : (ts + 1) * P],
                        wo_sb[:, ic, :],
                        start=(ic == 0),
                        stop=(ic == NIC - 1),
                    )
                nc.vector.tensor_add(out=y_g[:, ts, :], in0=pf, in1=bo_bc)

            nc.sync.dma_start(
                out=y_d[tok, :].rearrange("(t p) q -> p t q", p=P), in_=y_g
            )

        pending = None
        for g in range(groups):
            exp_g = emit_front(g)
            if pending is not None:
                emit_back(pending[0], pending[1])
            pending = (g, exp_g)
        emit_back(pending[0], pending[1])

    nc.compile()
    return nc


_CACHE = {}


def _get_nc():
    if "nc" not in _CACHE:
        _CACHE["nc"] = build_kernel()
    return _CACHE["nc"]


def run(inputs, trace=False, **kw):
    nc = _get_nc()
    in_maps = []
    for i in range(B):
        m = {
            "x": np.asarray(inputs["x"][i], dtype=np.float32),
            "context": np.asarray(inputs["context"][i], dtype=np.float32),
            "Wq": np.asarray(inputs["Wq"], dtype=np.float32),
            "Wk": np.asarray(inputs["Wk"], dtype=np.float32),
            "Wv": np.asarray(inputs["Wv"], dtype=np.float32),
            "Wo": np.asarray(inputs["Wo"], dtype=np.float32),
            "bo": np.asarray(inputs["bo"], dtype=np.float32),
        }
        in_maps.append(m)
    res = run_bass_kernel_spmd(nc, in_maps, list(range(B)), trace=trace, **kw)
    out = np.stack([res.results[i]["y"] for i in range(B)], axis=0)
    return out, res


def kernel(**inputs):
    out, _ = run(inputs)
    return out


# revision 11
# speedup vs baseline: 1.1151x; 1.0307x over previous
"""Cross-attention Trainium2 kernel (8-core data-parallel over batch).

Per-core computation (one batch element per NeuronCore):
  q = x @ Wq; k = ctx @ Wk; v = ctx @ Wv
  attn = softmax((q k^T) / sqrt(dh)); out = attn @ v; y = out @ Wo + bo

v2 structure (vs the v1 baseline):
  - weights arrive fp32 over the two HWDGE rings (sync/scalar) and are cast
    to bf16 on ACT/DVE, so the serial SWDGE (gpsimd cast-load) queue only
    carries the per-group x tiles -> the PE starts ~10us earlier.
  - xT via PE transposes + DVE copies (the DMA XBAR transpose path measured
    slower end-to-end: Tile serializes dma_start_transpose against all other
    in-flight DMAs as a HW-deadlock guard, which convoys the x/weight loads).
  - scores per head pair are emitted back-to-back as 64x128 row tiles
    (partition bases 0/64) into one 2-bank PSUM tile; the PE runs both
    concurrently (measured pair span ~320ns vs 432 serial), and a single
    ACT exp sweeps the 2-bank tile.
  - softmax denominators: col-tiled (128x64) pair matmuls with a shared
    [77->128, 64] all-ones bf16 stationary write sum_ctx(exp_h) broadcast
    across the 64 out partitions of each head; same tile mode as the
    attention-output pair matmuls (no extra PE mode switch).
  - attn-output pairs unchanged (col tiles 0/64), evicted with *1/rowsum.
  - y stores ride the sync HWDGE ring; everything on-chip stays bf16 for
    matmul operands with fp32 PSUM accumulation.
"""

import numpy as np

import concourse.bass as bass
import concourse.tile as tile
from concourse import bacc, mybir
from concourse.bass_utils import run_bass_kernel_spmd
from concourse.masks import make_identity

F32 = mybir.dt.float32
BF16 = mybir.dt.bfloat16

B, N, M = 8, 4096, 77
QD, CD, H, DH = 512, 768, 8, 64
INNER = H * DH  # 512
P = 128
S = 512  # token group size
NQC = QD // P  # 4 qd chunks
NCC = CD // P  # 6 cd chunks
NIC = INNER // P  # 4 inner chunks
NTS = S // P  # 4 token sub-tiles per group
SCALE = DH ** -0.5
MP = 128  # context length padded to full partition width (zeros are inert)


def build_kernel(groups: int = N // S):
    nc = bacc.Bacc(None, target_bir_lowering=False, debug=False)

    x_d = nc.dram_tensor("x", [N, QD], F32, kind="ExternalInput")
    ctx_d = nc.dram_tensor("context", [M, CD], F32, kind="ExternalInput")
    wq_d = nc.dram_tensor("Wq", [QD, INNER], F32, kind="ExternalInput")
    wk_d = nc.dram_tensor("Wk", [CD, INNER], F32, kind="ExternalInput")
    wv_d = nc.dram_tensor("Wv", [CD, INNER], F32, kind="ExternalInput")
    wo_d = nc.dram_tensor("Wo", [INNER, QD], F32, kind="ExternalInput")
    bo_d = nc.dram_tensor("bo", [QD], F32, kind="ExternalInput")
    y_d = nc.dram_tensor("y", [N, QD], F32, kind="ExternalOutput")

    from contextlib import ExitStack

    with tile.TileContext(nc) as tc, ExitStack() as st:
        consts = st.enter_context(tc.tile_pool(name="consts", bufs=1))
        kvp = st.enter_context(tc.tile_pool(name="kv", bufs=1))
        xin = st.enter_context(tc.tile_pool(name="xin", bufs=3))
        xtp = st.enter_context(tc.tile_pool(name="xt", bufs=2))
        qtp = st.enter_context(tc.tile_pool(name="qt", bufs=2))
        expp = st.enter_context(tc.tile_pool(name="expp", bufs=2))
        rcp = st.enter_context(tc.tile_pool(name="rcp", bufs=2))
        outp = st.enter_context(tc.tile_pool(name="outp", bufs=2))
        yp = st.enter_context(tc.tile_pool(name="yp", bufs=2))

        # PSUM: 8 banks. qf 2x[128,512] (2) + s 2x[128,2,512] (4) + r 2x[128,512] (2)
        ps_qf = st.enter_context(tc.tile_pool(name="ps_qf", bufs=2, space="PSUM"))
        ps_s = st.enter_context(tc.tile_pool(name="ps_s", bufs=2, space="PSUM"))
        ps_r = st.enter_context(tc.tile_pool(name="ps_r", bufs=2, space="PSUM"))

        # ---- tiny constants (engines only, no DMA) ------------------------------
        identity = consts.tile([P, P], BF16)
        make_identity(nc, identity)

        # all-ones [77, 64] stationary (zero-padded to 128 rows) for the
        # col-tiled rowsum matmuls
        ones64 = consts.tile([MP, DH], BF16)
        nc.vector.memset(ones64, 0.0)
        nc.vector.memset(ones64[:M, :], 1.0)

        # ---- loads: one serial SWDGE (gpsimd cast-load) queue, in need order ----
        # A single queue keeps DMA completion order == consumption order; the
        # HWDGE-parallel variant measured worse (SDMA round-robins the rings,
        # so the early-critical small loads finish last).
        def load_x(g):
            x_g = xin.tile([P, NTS, QD], BF16)
            nc.gpsimd.dma_start(
                out=x_g,
                in_=x_d[g * S : (g + 1) * S, :].rearrange("(t p) q -> p t q", p=P),
            )
            return x_g

        ctx_sb = kvp.tile([MP, CD], BF16)
        nc.vector.memset(ctx_sb, 0.0)
        nc.gpsimd.dma_start(out=ctx_sb[:M, :], in_=ctx_d[:, :])

        x_pre = [load_x(0)]

        wk_sb = consts.tile([P, NCC, INNER], BF16)
        nc.gpsimd.dma_start(
            out=wk_sb, in_=wk_d.ap().rearrange("(c p) n -> p c n", p=P)
        )

        wq_sb = consts.tile([P, NQC, INNER], BF16)
        nc.gpsimd.dma_start(
            out=wq_sb, in_=wq_d.ap().rearrange("(c p) n -> p c n", p=P)
        )

        x_pre.append(load_x(1))

        wv_sb = consts.tile([P, NCC, INNER], BF16)
        nc.gpsimd.dma_start(
            out=wv_sb, in_=wv_d.ap().rearrange("(c p) n -> p c n", p=P)
        )

        wo_sb = consts.tile([P, NIC, QD], BF16)
        nc.gpsimd.dma_start(
            out=wo_sb, in_=wo_d.ap().rearrange("(c p) n -> p c n", p=P)
        )

        bo_bc = consts.tile([P, QD], F32)
        bo_ap = bo_d.ap()
        nc.gpsimd.dma_start(
            out=bo_bc, in_=bass.AP(bo_ap.tensor, bo_ap.offset, [[0, P], [1, QD]])
        )

        # PE warmup: dummy transposes while the first DMAs are in flight keep
        # the HAM clock gate at 8/8 so the real prologue matmuls run at 2.4GHz
        for _ in range(24):
            ptw = ps_r.tile([P, P], BF16, tag="ps_r")
            nc.tensor.transpose(ptw, identity, identity)

        # ---- context projections (tiny) -----------------------------------------
        ctxT = kvp.tile([P, NCC, MP], BF16)
        for cc in range(NCC):
            pt = ps_r.tile([P, MP], BF16, tag="ps_r")
            nc.tensor.transpose(pt, ctx_sb[:, cc * P : (cc + 1) * P], identity)
            nc.vector.tensor_copy(out=ctxT[:, cc, :], in_=pt)

        kT = kvp.tile([P, NIC, MP], BF16)
        for ic in range(NIC):
            pk = ps_qf.tile([P, MP], F32, tag="ps_qf")
            for cc in range(NCC):
                nc.tensor.matmul(
                    pk,
                    wk_sb[:, cc, ic * P : (ic + 1) * P],
                    ctxT[:, cc, :],
                    start=(cc == 0),
                    stop=(cc == NCC - 1),
                )
            nc.vector.tensor_copy(out=kT[:, ic, :], in_=pk)

        v_sb = kvp.tile([MP, INNER], BF16)
        pv = ps_s.tile([MP, INNER], F32, tag="ps_s")
        for cc in range(NCC):
            nc.tensor.matmul(
                pv,
                ctxT[:, cc, :],
                wv_sb[:, cc, :],
                start=(cc == 0),
                stop=(cc == NCC - 1),
            )
        nc.vector.tensor_copy(out=v_sb, in_=pv)

        # ---- main loop over token groups ----------------------------------------
        # Software-pipelined emission: group g's rowsums / attention-output /
        # final projection are emitted one iteration later, after group g+1's
        # front block, so the scheduler can fill PE stalls (exp latency)
        # with back-phase matmuls.

        def emit_front(g):
            x_g = x_pre[g]
            if g + 2 < groups:
                x_pre.append(load_x(g + 2))

            # transpose x tiles: xT[p, c, t*128+j] = x[t*128+..., c*128+p];
            # 4 PE transposes land in one psum bank, one DVE copy per chunk
            xT = xtp.tile([P, NQC, S], BF16)
            for c in range(NQC):
                pt = ps_r.tile([P, S], BF16, tag="ps_r")
                for ts in range(NTS):
                    nc.tensor.transpose(
                        pt[:, ts * P : (ts + 1) * P],
                        x_g[:, ts, c * P : (c + 1) * P],
                        identity,
                    )
                nc.vector.tensor_copy(out=xT[:, c, :], in_=pt)

            # qT[inner, tok]
            qT = qtp.tile([P, NIC, S], BF16)
            for ic in range(NIC):
                pq = ps_qf.tile([P, S], F32, tag="ps_qf")
                for c in range(NQC):
                    nc.tensor.matmul(
                        pq,
                        wq_sb[:, c, ic * P : (ic + 1) * P],
                        xT[:, c, :],
                        start=(c == 0),
                        stop=(c == NQC - 1),
                    )
                nc.scalar.copy(out=qT[:, ic, :], in_=pq)

            # scores -> exp per head pair: the pair runs as concurrent 64x128
            # row tiles (partition bases 0/64) into one 2-bank psum tile, then
            # one ACT exp sweeps both banks.
            exp_g = expp.tile([MP, H, S], BF16)
            for pp in range(H // 2):
                sp = ps_s.tile([MP, 2, S], F32, tag="ps_s")
                nc.tensor.matmul(
                    sp[:, 0, :],
                    kT[0:DH, pp, :],
                    qT[0:DH, pp, :],
                    start=True,
                    stop=True,
                )
                nc.tensor.matmul(
                    sp[:, 1, :],
                    kT[DH : 2 * DH, pp, :],
                    qT[DH : 2 * DH, pp, :],
                    start=True,
                    stop=True,
                )
                nc.scalar.activation(
                    out=exp_g[:, 2 * pp : 2 * pp + 2, :],
                    in_=sp,
                    func=mybir.ActivationFunctionType.Exp,
                    scale=SCALE,
                )
            return exp_g

        def emit_back(g, exp_g):
            # rowsums, broadcast across each head's 64 out partitions by the
            # col-tiled ones matmuls; then reciprocal per pair
            rec_g = rcp.tile([P, H // 2, S], F32)
            for pp in range(H // 2):
                pr = ps_r.tile([P, S], F32, tag="ps_r")
                nc.tensor.matmul(
                    pr[0:DH, :],
                    ones64,
                    exp_g[:, 2 * pp, :],
                    start=True,
                    stop=True,
                    tile_position=(0, 0),
                )
                nc.tensor.matmul(
                    pr[DH : 2 * DH, :],
                    ones64,
                    exp_g[:, 2 * pp + 1, :],
                    start=True,
                    stop=True,
                    tile_position=(0, DH),
                )
                nc.vector.reciprocal_approx_fast(out=rec_g[:, pp, :], in_=pr)

            # outT (unnormalized) * (1/r); pair-packed into one bank
            outT = outp.tile([P, NIC, S], BF16)
            for pp in range(H // 2):
                po = ps_r.tile([P, S], F32, tag="ps_r")
                for side in range(2):
                    h = 2 * pp + side
                    nc.tensor.matmul(
                        po[side * DH : (side + 1) * DH, :],
                        v_sb[:, h * DH : (h + 1) * DH],
                        exp_g[:, h, :],
                        start=True,
                        stop=True,
                        tile_position=(0, side * DH),
                    )
                nc.vector.tensor_mul(
                    out=outT[:, pp, :], in0=po, in1=rec_g[:, pp, :]
                )

            # final projection + bias; the last group streams each token
            # sub-tile out as soon as its bias-add lands (shorter tail)
            tok = slice(g * S, (g + 1) * S)
            y_g = yp.tile([P, NTS, QD], F32)
            y_view = y_d[tok, :].rearrange("(t p) q -> p t q", p=P)
            last = g == groups - 1
            for ts in range(NTS):
                pf = ps_qf.tile([P, QD], F32, tag="ps_qf")
                for ic in range(NIC):
                    nc.tensor.matmul(
                        pf,
                        outT[:, ic, ts * P : (ts + 1) * P],
                        wo_sb[:, ic, :],
                        start=(ic == 0),
                        stop=(ic == NIC - 1),
                    )
                nc.vector.tensor_add(out=y_g[:, ts, :], in0=pf, in1=bo_bc)
                if last:
                    nc.sync.dma_start(out=y_view[:, ts, :], in_=y_g[:, ts, :])
            if not last:
                nc.sync.dma_start(out=y_view, in_=y_g)

        pending = None
        for g in range(groups):
            exp_g = emit_front(g)
            if pending is not None:
                emit_back(pending[0], pending[1])
            pending = (g, exp_g)
        emit_back(pending[0], pending[1])

    nc.compile()
    return nc


_CACHE = {}


def _get_nc():
    if "nc" not in _CACHE:
        _CACHE["nc"] = build_kernel()
    return _CACHE["nc"]


def run(inputs, trace=False, **kw):
    nc = _get_nc()
    in_maps = []
    for i in range(B):
        m = {
            "x": np.asarray(inputs["x"][i], dtype=np.float32),
            "context": np.asarray(inputs["context"][i], dtype=np.float32),
            "Wq": np.asarray(inputs["Wq"], dtype=np.float32),
            "Wk": np.asarray(inputs["Wk"], dtype=np.float32),
            "Wv": np.asarray(inputs["Wv"], dtype=np.float32),
            "Wo": np.asarray(inputs["Wo"], dtype=np.float32),
            "bo": np.asarray(inputs["bo"], dtype=np.float32),
        }
        in_maps.append(m)
    res = run_bass_kernel_spmd(nc, in_maps, list(range(B)), trace=trace, **kw)
    out = np.stack([res.results[i]["y"] for i in range(B)], axis=0)
    return out, res


def kernel(**inputs):
    out, _ = run(inputs)
    return out


# revision 15
# speedup vs baseline: 1.1354x; 1.0182x over previous
"""Cross-attention Trainium2 kernel (8-core data-parallel over batch).

Per-core computation (one batch element per NeuronCore):
  q = x @ Wq; k = ctx @ Wk; v = ctx @ Wv
  attn = softmax((q k^T) / sqrt(dh)); out = attn @ v; y = out @ Wo + bo

v2 structure (vs the v1 baseline):
  - weights arrive fp32 over the two HWDGE rings (sync/scalar) and are cast
    to bf16 on ACT/DVE, so the serial SWDGE (gpsimd cast-load) queue only
    carries the per-group x tiles -> the PE starts ~10us earlier.
  - xT via PE transposes + DVE copies (the DMA XBAR transpose path measured
    slower end-to-end: Tile serializes dma_start_transpose against all other
    in-flight DMAs as a HW-deadlock guard, which convoys the x/weight loads).
  - scores per head pair are emitted back-to-back as 64x128 row tiles
    (partition bases 0/64) into one 2-bank PSUM tile; the PE runs both
    concurrently (measured pair span ~320ns vs 432 serial), and a single
    ACT exp sweeps the 2-bank tile.
  - softmax denominators: col-tiled (128x64) pair matmuls with a shared
    [77->128, 64] all-ones bf16 stationary write sum_ctx(exp_h) broadcast
    across the 64 out partitions of each head; same tile mode as the
    attention-output pair matmuls (no extra PE mode switch).
  - attn-output pairs unchanged (col tiles 0/64), evicted with *1/rowsum.
  - y stores ride the sync HWDGE ring; everything on-chip stays bf16 for
    matmul operands with fp32 PSUM accumulation.
"""

import numpy as np

import concourse.bass as bass
import concourse.tile as tile
from concourse import bacc, mybir
from concourse.bass_utils import run_bass_kernel_spmd
from concourse.masks import make_identity

F32 = mybir.dt.float32
BF16 = mybir.dt.bfloat16

B, N, M = 8, 4096, 77
QD, CD, H, DH = 512, 768, 8, 64
INNER = H * DH  # 512
P = 128
S = 512  # token group size
NQC = QD // P  # 4 qd chunks
NCC = CD // P  # 6 cd chunks
NIC = INNER // P  # 4 inner chunks
NTS = S // P  # 4 token sub-tiles per group
SCALE = DH ** -0.5
MP = 128  # context length padded to full partition width (zeros are inert)


def build_kernel(groups: int = N // S):
    nc = bacc.Bacc(None, target_bir_lowering=False, debug=False)

    x_d = nc.dram_tensor("x", [N, QD], F32, kind="ExternalInput")
    ctx_d = nc.dram_tensor("context", [M, CD], F32, kind="ExternalInput")
    wq_d = nc.dram_tensor("Wq", [QD, INNER], F32, kind="ExternalInput")
    wk_d = nc.dram_tensor("Wk", [CD, INNER], F32, kind="ExternalInput")
    wv_d = nc.dram_tensor("Wv", [CD, INNER], F32, kind="ExternalInput")
    wo_d = nc.dram_tensor("Wo", [INNER, QD], F32, kind="ExternalInput")
    bo_d = nc.dram_tensor("bo", [QD], F32, kind="ExternalInput")
    y_d = nc.dram_tensor("y", [N, QD], F32, kind="ExternalOutput")

    from contextlib import ExitStack

    with tile.TileContext(nc) as tc, ExitStack() as st:
        consts = st.enter_context(tc.tile_pool(name="consts", bufs=1))
        kvp = st.enter_context(tc.tile_pool(name="kv", bufs=1))
        xin = st.enter_context(tc.tile_pool(name="xin", bufs=3))
        xtp = st.enter_context(tc.tile_pool(name="xt", bufs=2))
        qtp = st.enter_context(tc.tile_pool(name="qt", bufs=2))
        expp = st.enter_context(tc.tile_pool(name="expp", bufs=3))
        rcp = st.enter_context(tc.tile_pool(name="rcp", bufs=2))
        outp = st.enter_context(tc.tile_pool(name="outp", bufs=3))
        yp = st.enter_context(tc.tile_pool(name="yp", bufs=2))

        # PSUM: 8 banks. qf 2x[128,512] (2) + s 2x[128,2,512] (4) + r 2x[128,512] (2)
        ps_qf = st.enter_context(tc.tile_pool(name="ps_qf", bufs=2, space="PSUM"))
        ps_s = st.enter_context(tc.tile_pool(name="ps_s", bufs=2, space="PSUM"))
        ps_r = st.enter_context(tc.tile_pool(name="ps_r", bufs=2, space="PSUM"))

        # ---- tiny constants (engines only, no DMA) ------------------------------
        identity = consts.tile([P, P], BF16)
        make_identity(nc, identity)

        # all-ones [77, 64] stationary (zero-padded to 128 rows) for the
        # col-tiled rowsum matmuls
        ones64 = consts.tile([MP, DH], BF16)
        nc.vector.memset(ones64, 0.0)
        nc.vector.memset(ones64[:M, :], 1.0)

        # ---- loads: one serial SWDGE (gpsimd cast-load) queue, in need order ----
        # A single queue keeps DMA completion order == consumption order; the
        # HWDGE-parallel variant measured worse (SDMA round-robins the rings,
        # so the early-critical small loads finish last).
        def load_x(g):
            x_g = xin.tile([P, NTS, QD], BF16)
            nc.gpsimd.dma_start(
                out=x_g,
                in_=x_d[g * S : (g + 1) * S, :].rearrange("(t p) q -> p t q", p=P),
            )
            return x_g

        ctx_sb = kvp.tile([MP, CD], BF16)
        nc.vector.memset(ctx_sb, 0.0)
        nc.gpsimd.dma_start(out=ctx_sb[:M, :], in_=ctx_d[:, :])

        x_pre = [load_x(0)]

        wk_sb = consts.tile([P, NCC, INNER], BF16)
        nc.gpsimd.dma_start(
            out=wk_sb, in_=wk_d.ap().rearrange("(c p) n -> p c n", p=P)
        )

        wq_sb = consts.tile([P, NQC, INNER], BF16)
        nc.gpsimd.dma_start(
            out=wq_sb, in_=wq_d.ap().rearrange("(c p) n -> p c n", p=P)
        )

        x_pre.append(load_x(1))

        wv_sb = consts.tile([P, NCC, INNER], BF16)
        nc.gpsimd.dma_start(
            out=wv_sb, in_=wv_d.ap().rearrange("(c p) n -> p c n", p=P)
        )

        wo_sb = consts.tile([P, NIC, QD], BF16)
        nc.gpsimd.dma_start(
            out=wo_sb, in_=wo_d.ap().rearrange("(c p) n -> p c n", p=P)
        )

        bo_bc = consts.tile([P, QD], F32)
        bo_ap = bo_d.ap()
        nc.gpsimd.dma_start(
            out=bo_bc, in_=bass.AP(bo_ap.tensor, bo_ap.offset, [[0, P], [1, QD]])
        )

        # PE warmup: dummy transposes while the first DMAs are in flight keep
        # the HAM clock gate at 8/8 so the real prologue matmuls run at 2.4GHz
        for _ in range(24):
            ptw = ps_r.tile([P, P], BF16, tag="ps_r")
            nc.tensor.transpose(ptw, identity, identity)

        # ---- context projections (tiny) -----------------------------------------
        ctxT = kvp.tile([P, NCC, MP], BF16)
        for cc in range(NCC):
            pt = ps_r.tile([P, MP], BF16, tag="ps_r")
            nc.tensor.transpose(pt, ctx_sb[:, cc * P : (cc + 1) * P], identity)
            nc.vector.tensor_copy(out=ctxT[:, cc, :], in_=pt)

        kT = kvp.tile([P, NIC, MP], BF16)
        for ic in range(NIC):
            pk = ps_qf.tile([P, MP], F32, tag="ps_qf")
            for cc in range(NCC):
                nc.tensor.matmul(
                    pk,
                    wk_sb[:, cc, ic * P : (ic + 1) * P],
                    ctxT[:, cc, :],
                    start=(cc == 0),
                    stop=(cc == NCC - 1),
                )
            nc.vector.tensor_copy(out=kT[:, ic, :], in_=pk)

        v_sb = kvp.tile([MP, INNER], BF16)
        pv = ps_s.tile([MP, INNER], F32, tag="ps_s")
        for cc in range(NCC):
            nc.tensor.matmul(
                pv,
                ctxT[:, cc, :],
                wv_sb[:, cc, :],
                start=(cc == 0),
                stop=(cc == NCC - 1),
            )
        nc.vector.tensor_copy(out=v_sb, in_=pv)

        # ---- main loop over token groups ----------------------------------------
        # Software-pipelined emission: group g's rowsums / attention-output /
        # final projection are emitted one iteration later, after group g+1's
        # front block, so the scheduler can fill PE stalls (exp latency)
        # with back-phase matmuls.

        def emit_front(g):
            x_g = x_pre[g]
            if g + 2 < groups:
                x_pre.append(load_x(g + 2))

            # transpose x tiles: xT[p, c, t*128+j] = x[t*128+..., c*128+p];
            # 4 PE transposes land in one psum bank, one DVE copy per chunk
            xT = xtp.tile([P, NQC, S], BF16)
            for c in range(NQC):
                pt = ps_r.tile([P, S], BF16, tag="ps_r")
                for ts in range(NTS):
                    nc.tensor.transpose(
                        pt[:, ts * P : (ts + 1) * P],
                        x_g[:, ts, c * P : (c + 1) * P],
                        identity,
                    )
                nc.vector.tensor_copy(out=xT[:, c, :], in_=pt)

            # qT[inner, tok]
            qT = qtp.tile([P, NIC, S], BF16)
            for ic in range(NIC):
                pq = ps_qf.tile([P, S], F32, tag="ps_qf")
                for c in range(NQC):
                    nc.tensor.matmul(
                        pq,
                        wq_sb[:, c, ic * P : (ic + 1) * P],
                        xT[:, c, :],
                        start=(c == 0),
                        stop=(c == NQC - 1),
                    )
                nc.scalar.copy(out=qT[:, ic, :], in_=pq)

            # scores -> exp per head pair: the pair runs as concurrent 64x128
            # row tiles (partition bases 0/64) into one 2-bank psum tile, then
            # one ACT exp sweeps both banks.
            exp_g = expp.tile([MP, H, S], BF16)
            for pp in range(H // 2):
                sp = ps_s.tile([MP, 2, S], F32, tag="ps_s")
                nc.tensor.matmul(
                    sp[:, 0, :],
                    kT[0:DH, pp, :],
                    qT[0:DH, pp, :],
                    start=True,
                    stop=True,
                )
                nc.tensor.matmul(
                    sp[:, 1, :],
                    kT[DH : 2 * DH, pp, :],
                    qT[DH : 2 * DH, pp, :],
                    start=True,
                    stop=True,
                )
                nc.scalar.activation(
                    out=exp_g[:, 2 * pp : 2 * pp + 2, :],
                    in_=sp,
                    func=mybir.ActivationFunctionType.Exp,
                    scale=SCALE,
                )
            return exp_g

        def emit_attn(g, exp_g):
            # rowsums, broadcast across each head's 64 out partitions by the
            # col-tiled ones matmuls; then reciprocal per pair
            rec_g = rcp.tile([P, H // 2, S], F32)
            for pp in range(H // 2):
                pr = ps_r.tile([P, S], F32, tag="ps_r")
                nc.tensor.matmul(
                    pr[0:DH, :],
                    ones64,
                    exp_g[:, 2 * pp, :],
                    start=True,
                    stop=True,
                    tile_position=(0, 0),
                )
                nc.tensor.matmul(
                    pr[DH : 2 * DH, :],
                    ones64,
                    exp_g[:, 2 * pp + 1, :],
                    start=True,
                    stop=True,
                    tile_position=(0, DH),
                )
                nc.vector.reciprocal_approx_fast(out=rec_g[:, pp, :], in_=pr)

            # outT (unnormalized) * (1/r); pair-packed into one bank
            outT = outp.tile([P, NIC, S], BF16)
            for pp in range(H // 2):
                po = ps_r.tile([P, S], F32, tag="ps_r")
                for side in range(2):
                    h = 2 * pp + side
                    nc.tensor.matmul(
                        po[side * DH : (side + 1) * DH, :],
                        v_sb[:, h * DH : (h + 1) * DH],
                        exp_g[:, h, :],
                        start=True,
                        stop=True,
                        tile_position=(0, side * DH),
                    )
                nc.vector.tensor_mul(
                    out=outT[:, pp, :], in0=po, in1=rec_g[:, pp, :]
                )
            return outT

        def emit_proj(g, outT):
            # final projection + bias; the last group streams each token
            # sub-tile out as soon as its bias-add lands (shorter tail)
            tok = slice(g * S, (g + 1) * S)
            y_g = yp.tile([P, NTS, QD], F32)
            y_view = y_d[tok, :].rearrange("(t p) q -> p t q", p=P)
            last = g == groups - 1
            for ts in range(NTS):
                pf = ps_qf.tile([P, QD], F32, tag="ps_qf")
                for ic in range(NIC):
                    nc.tensor.matmul(
                        pf,
                        outT[:, ic, ts * P : (ts + 1) * P],
                        wo_sb[:, ic, :],
                        start=(ic == 0),
                        stop=(ic == NIC - 1),
                    )
                nc.vector.tensor_add(out=y_g[:, ts, :], in0=pf, in1=bo_bc)
                if last:
                    nc.sync.dma_start(out=y_view[:, ts, :], in_=y_g[:, ts, :])
            if not last:
                nc.sync.dma_start(out=y_view, in_=y_g)

        # pipeline: front(g) | attn(g-1) | proj(g-2)
        attn_q = []  # (g, exp_g) awaiting attn
        proj_q = []  # (g, outT) awaiting final projection
        for g in range(groups):
            exp_g = emit_front(g)
            if attn_q:
                ga, ea = attn_q.pop(0)
                proj_q.append((ga, emit_attn(ga, ea)))
            if len(proj_q) > 1 or g == groups - 1:
                gp, op_ = proj_q.pop(0)
                emit_proj(gp, op_)
            attn_q.append((g, exp_g))
        while attn_q:
            ga, ea = attn_q.pop(0)
            proj_q.append((ga, emit_attn(ga, ea)))
        while proj_q:
            gp, op_ = proj_q.pop(0)
            emit_proj(gp, op_)

    nc.compile()
    return nc


_CACHE = {}


def _get_nc():
    if "nc" not in _CACHE:
        _CACHE["nc"] = build_kernel()
    return _CACHE["nc"]


def run(inputs, trace=False, **kw):
    nc = _get_nc()
    in_maps = []
    for i in range(B):
        m = {
            "x": np.asarray(inputs["x"][i], dtype=np.float32),
            "context": np.asarray(inputs["context"][i], dtype=np.float32),
            "Wq": np.asarray(inputs["Wq"], dtype=np.float32),
            "Wk": np.asarray(inputs["Wk"], dtype=np.float32),
            "Wv": np.asarray(inputs["Wv"], dtype=np.float32),
            "Wo": np.asarray(inputs["Wo"], dtype=np.float32),
            "bo": np.asarray(inputs["bo"], dtype=np.float32),
        }
        in_maps.append(m)
    res = run_bass_kernel_spmd(nc, in_maps, list(range(B)), trace=trace, **kw)
    out = np.stack([res.results[i]["y"] for i in range(B)], axis=0)
    return out, res


def kernel(**inputs):
    out, _ = run(inputs)
    return out


# revision 18
# speedup vs baseline: 1.1566x; 1.0187x over previous
"""Cross-attention Trainium2 kernel (8-core data-parallel over batch).

Per-core computation (one batch element per NeuronCore):
  q = x @ Wq; k = ctx @ Wk; v = ctx @ Wv
  attn = softmax((q k^T) / sqrt(dh)); out = attn @ v; y = out @ Wo + bo

v2 structure (vs the v1 baseline):
  - weights arrive fp32 over the two HWDGE rings (sync/scalar) and are cast
    to bf16 on ACT/DVE, so the serial SWDGE (gpsimd cast-load) queue only
    carries the per-group x tiles -> the PE starts ~10us earlier.
  - xT via PE transposes + DVE copies (the DMA XBAR transpose path measured
    slower end-to-end: Tile serializes dma_start_transpose against all other
    in-flight DMAs as a HW-deadlock guard, which convoys the x/weight loads).
  - scores per head pair are emitted back-to-back as 64x128 row tiles
    (partition bases 0/64) into one 2-bank PSUM tile; the PE runs both
    concurrently (measured pair span ~320ns vs 432 serial), and a single
    ACT exp sweeps the 2-bank tile.
  - softmax denominators: col-tiled (128x64) pair matmuls with a shared
    [77->128, 64] all-ones bf16 stationary write sum_ctx(exp_h) broadcast
    across the 64 out partitions of each head; same tile mode as the
    attention-output pair matmuls (no extra PE mode switch).
  - attn-output pairs unchanged (col tiles 0/64), evicted with *1/rowsum.
  - y stores ride the sync HWDGE ring; everything on-chip stays bf16 for
    matmul operands with fp32 PSUM accumulation.
"""

import numpy as np

import concourse.bass as bass
import concourse.tile as tile
from concourse import bacc, mybir
from concourse.bass_utils import run_bass_kernel_spmd
from concourse.masks import make_identity

F32 = mybir.dt.float32
BF16 = mybir.dt.bfloat16

B, N, M = 8, 4096, 77
QD, CD, H, DH = 512, 768, 8, 64
INNER = H * DH  # 512
P = 128
S = 512  # token group size
NQC = QD // P  # 4 qd chunks
NCC = CD // P  # 6 cd chunks
NIC = INNER // P  # 4 inner chunks
NTS = S // P  # 4 token sub-tiles per group
SCALE = DH ** -0.5
MP = 128  # context length padded to full partition width (zeros are inert)


def build_kernel(groups: int = N // S):
    nc = bacc.Bacc(None, target_bir_lowering=False, debug=False)

    x_d = nc.dram_tensor("x", [N, QD], F32, kind="ExternalInput")
    ctx_d = nc.dram_tensor("context", [M, CD], F32, kind="ExternalInput")
    wq_d = nc.dram_tensor("Wq", [QD, INNER], F32, kind="ExternalInput")
    wk_d = nc.dram_tensor("Wk", [CD, INNER], F32, kind="ExternalInput")
    wv_d = nc.dram_tensor("Wv", [CD, INNER], F32, kind="ExternalInput")
    wo_d = nc.dram_tensor("Wo", [INNER, QD], F32, kind="ExternalInput")
    bo_d = nc.dram_tensor("bo", [QD], F32, kind="ExternalInput")
    y_d = nc.dram_tensor("y", [N, QD], BF16, kind="ExternalOutput")

    from contextlib import ExitStack

    with tile.TileContext(nc) as tc, ExitStack() as st:
        consts = st.enter_context(tc.tile_pool(name="consts", bufs=1))
        kvp = st.enter_context(tc.tile_pool(name="kv", bufs=1))
        xin = st.enter_context(tc.tile_pool(name="xin", bufs=3))
        xtp = st.enter_context(tc.tile_pool(name="xt", bufs=2))
        qtp = st.enter_context(tc.tile_pool(name="qt", bufs=2))
        expp = st.enter_context(tc.tile_pool(name="expp", bufs=3))
        rcp = st.enter_context(tc.tile_pool(name="rcp", bufs=2))
        outp = st.enter_context(tc.tile_pool(name="outp", bufs=3))
        yp = st.enter_context(tc.tile_pool(name="yp", bufs=2))

        # PSUM: 8 banks. qf 2x[128,512] (2) + s 2x[128,2,512] (4) + r 2x[128,512] (2)
        ps_qf = st.enter_context(tc.tile_pool(name="ps_qf", bufs=2, space="PSUM"))
        ps_s = st.enter_context(tc.tile_pool(name="ps_s", bufs=2, space="PSUM"))
        ps_r = st.enter_context(tc.tile_pool(name="ps_r", bufs=2, space="PSUM"))

        # ---- tiny constants (engines only, no DMA) ------------------------------
        identity = consts.tile([P, P], BF16)
        make_identity(nc, identity)

        # all-ones [77, 64] stationary (zero-padded to 128 rows) for the
        # col-tiled rowsum matmuls
        ones64 = consts.tile([MP, DH], BF16)
        nc.vector.memset(ones64, 0.0)
        nc.vector.memset(ones64[:M, :], 1.0)

        # ---- loads: one serial SWDGE (gpsimd cast-load) queue, in need order ----
        # A single queue keeps DMA completion order == consumption order; the
        # HWDGE-parallel variant measured worse (SDMA round-robins the rings,
        # so the early-critical small loads finish last).
        def load_x(g):
            x_g = xin.tile([P, NTS, QD], BF16)
            nc.gpsimd.dma_start(
                out=x_g,
                in_=x_d[g * S : (g + 1) * S, :].rearrange("(t p) q -> p t q", p=P),
            )
            return x_g

        ctx_sb = kvp.tile([MP, CD], BF16)
        nc.vector.memset(ctx_sb, 0.0)
        nc.gpsimd.dma_start(out=ctx_sb[:M, :], in_=ctx_d[:, :])

        x_pre = [load_x(0)]

        wk_sb = consts.tile([P, NCC, INNER], BF16)
        nc.gpsimd.dma_start(
            out=wk_sb, in_=wk_d.ap().rearrange("(c p) n -> p c n", p=P)
        )

        wq_sb = consts.tile([P, NQC, INNER], BF16)
        nc.gpsimd.dma_start(
            out=wq_sb, in_=wq_d.ap().rearrange("(c p) n -> p c n", p=P)
        )

        x_pre.append(load_x(1))

        wv_sb = consts.tile([P, NCC, INNER], BF16)
        nc.gpsimd.dma_start(
            out=wv_sb, in_=wv_d.ap().rearrange("(c p) n -> p c n", p=P)
        )

        wo_sb = consts.tile([P, NIC, QD], BF16)
        nc.gpsimd.dma_start(
            out=wo_sb, in_=wo_d.ap().rearrange("(c p) n -> p c n", p=P)
        )

        bo_bc = consts.tile([P, QD], F32)
        bo_ap = bo_d.ap()
        nc.gpsimd.dma_start(
            out=bo_bc, in_=bass.AP(bo_ap.tensor, bo_ap.offset, [[0, P], [1, QD]])
        )

        # PE warmup: dummy transposes while the first DMAs are in flight keep
        # the HAM clock gate at 8/8 so the real prologue matmuls run at 2.4GHz
        for _ in range(24):
            ptw = ps_r.tile([P, P], BF16, tag="ps_r")
            nc.tensor.transpose(ptw, identity, identity)

        # ---- context projections (tiny) -----------------------------------------
        ctxT = kvp.tile([P, NCC, MP], BF16)
        for cc in range(NCC):
            pt = ps_r.tile([P, MP], BF16, tag="ps_r")
            nc.tensor.transpose(pt, ctx_sb[:, cc * P : (cc + 1) * P], identity)
            nc.vector.tensor_copy(out=ctxT[:, cc, :], in_=pt)

        kT = kvp.tile([P, NIC, MP], BF16)
        for ic in range(NIC):
            pk = ps_qf.tile([P, MP], F32, tag="ps_qf")
            for cc in range(NCC):
                nc.tensor.matmul(
                    pk,
                    wk_sb[:, cc, ic * P : (ic + 1) * P],
                    ctxT[:, cc, :],
                    start=(cc == 0),
                    stop=(cc == NCC - 1),
                )
            nc.vector.tensor_copy(out=kT[:, ic, :], in_=pk)

        v_sb = kvp.tile([MP, INNER], BF16)
        pv = ps_s.tile([MP, INNER], F32, tag="ps_s")
        for cc in range(NCC):
            nc.tensor.matmul(
                pv,
                ctxT[:, cc, :],
                wv_sb[:, cc, :],
                start=(cc == 0),
                stop=(cc == NCC - 1),
            )
        nc.vector.tensor_copy(out=v_sb, in_=pv)

        # ---- main loop over token groups ----------------------------------------
        # Software-pipelined emission: group g's rowsums / attention-output /
        # final projection are emitted one iteration later, after group g+1's
        # front block, so the scheduler can fill PE stalls (exp latency)
        # with back-phase matmuls.

        def emit_front(g):
            x_g = x_pre[g]
            if g + 2 < groups:
                x_pre.append(load_x(g + 2))

            # transpose x tiles: xT[p, c, t*128+j] = x[t*128+..., c*128+p];
            # 4 PE transposes land in one psum bank, one DVE copy per chunk
            xT = xtp.tile([P, NQC, S], BF16)
            for c in range(NQC):
                pt = ps_r.tile([P, S], BF16, tag="ps_r")
                for ts in range(NTS):
                    nc.tensor.transpose(
                        pt[:, ts * P : (ts + 1) * P],
                        x_g[:, ts, c * P : (c + 1) * P],
                        identity,
                    )
                nc.vector.tensor_copy(out=xT[:, c, :], in_=pt)

            # qT[inner, tok]
            qT = qtp.tile([P, NIC, S], BF16)
            for ic in range(NIC):
                pq = ps_qf.tile([P, S], F32, tag="ps_qf")
                for c in range(NQC):
                    nc.tensor.matmul(
                        pq,
                        wq_sb[:, c, ic * P : (ic + 1) * P],
                        xT[:, c, :],
                        start=(c == 0),
                        stop=(c == NQC - 1),
                    )
                nc.scalar.copy(out=qT[:, ic, :], in_=pq)

            # scores -> exp per head pair: the pair runs as concurrent 64x128
            # row tiles (partition bases 0/64) into one 2-bank psum tile, then
            # one ACT exp sweeps both banks.
            exp_g = expp.tile([MP, H, S], BF16)
            for pp in range(H // 2):
                sp = ps_s.tile([MP, 2, S], F32, tag="ps_s")
                nc.tensor.matmul(
                    sp[:, 0, :],
                    kT[0:DH, pp, :],
                    qT[0:DH, pp, :],
                    start=True,
                    stop=True,
                )
                nc.tensor.matmul(
                    sp[:, 1, :],
                    kT[DH : 2 * DH, pp, :],
                    qT[DH : 2 * DH, pp, :],
                    start=True,
                    stop=True,
                )
                nc.scalar.activation(
                    out=exp_g[:, 2 * pp : 2 * pp + 2, :],
                    in_=sp,
                    func=mybir.ActivationFunctionType.Exp,
                    scale=SCALE,
                )
            return exp_g

        def emit_attn(g, exp_g):
            # rowsums, broadcast across each head's 64 out partitions by the
            # col-tiled ones matmuls; then reciprocal per pair
            rec_g = rcp.tile([P, H // 2, S], F32)
            for pp in range(H // 2):
                pr = ps_r.tile([P, S], F32, tag="ps_r")
                nc.tensor.matmul(
                    pr[0:DH, :],
                    ones64,
                    exp_g[:, 2 * pp, :],
                    start=True,
                    stop=True,
                    tile_position=(0, 0),
                )
                nc.tensor.matmul(
                    pr[DH : 2 * DH, :],
                    ones64,
                    exp_g[:, 2 * pp + 1, :],
                    start=True,
                    stop=True,
                    tile_position=(0, DH),
                )
                nc.vector.reciprocal_approx_fast(out=rec_g[:, pp, :], in_=pr)

            # outT (unnormalized) * (1/r); pair-packed into one bank
            outT = outp.tile([P, NIC, S], BF16)
            for pp in range(H // 2):
                po = ps_r.tile([P, S], F32, tag="ps_r")
                for side in range(2):
                    h = 2 * pp + side
                    nc.tensor.matmul(
                        po[side * DH : (side + 1) * DH, :],
                        v_sb[:, h * DH : (h + 1) * DH],
                        exp_g[:, h, :],
                        start=True,
                        stop=True,
                        tile_position=(0, side * DH),
                    )
                nc.vector.tensor_mul(
                    out=outT[:, pp, :], in0=po, in1=rec_g[:, pp, :]
                )
            return outT

        def emit_proj(g, outT):
            # final projection + bias; the last group streams each token
            # sub-tile out as soon as its bias-add lands (shorter tail)
            tok = slice(g * S, (g + 1) * S)
            y_g = yp.tile([P, NTS, QD], BF16)
            y_view = y_d[tok, :].rearrange("(t p) q -> p t q", p=P)
            last = g == groups - 1
            for ts in range(NTS):
                pf = ps_qf.tile([P, QD], F32, tag="ps_qf")
                for ic in range(NIC):
                    nc.tensor.matmul(
                        pf,
                        outT[:, ic, ts * P : (ts + 1) * P],
                        wo_sb[:, ic, :],
                        start=(ic == 0),
                        stop=(ic == NIC - 1),
                    )
                nc.vector.tensor_add(out=y_g[:, ts, :], in0=pf, in1=bo_bc)
                if last:
                    nc.sync.dma_start(out=y_view[:, ts, :], in_=y_g[:, ts, :])
            if not last:
                nc.sync.dma_start(out=y_view, in_=y_g)

        # pipeline: front(g) | attn(g-1) | proj(g-2)
        attn_q = []  # (g, exp_g) awaiting attn
        proj_q = []  # (g, outT) awaiting final projection
        for g in range(groups):
            exp_g = emit_front(g)
            if attn_q:
                ga, ea = attn_q.pop(0)
                proj_q.append((ga, emit_attn(ga, ea)))
            if len(proj_q) > 1 or g == groups - 1:
                gp, op_ = proj_q.pop(0)
                emit_proj(gp, op_)
            attn_q.append((g, exp_g))
        while attn_q:
            ga, ea = attn_q.pop(0)
            proj_q.append((ga, emit_attn(ga, ea)))
        while proj_q:
            gp, op_ = proj_q.pop(0)
            emit_proj(gp, op_)

    nc.compile()
    return nc


_CACHE = {}


def _get_nc():
    if "nc" not in _CACHE:
        _CACHE["nc"] = build_kernel()
    return _CACHE["nc"]


def run(inputs, trace=False, **kw):
    nc = _get_nc()
    in_maps = []
    for i in range(B):
        m = {
            "x": np.asarray(inputs["x"][i], dtype=np.float32),
            "context": np.asarray(inputs["context"][i], dtype=np.float32),
            "Wq": np.asarray(inputs["Wq"], dtype=np.float32),
            "Wk": np.asarray(inputs["Wk"], dtype=np.float32),
            "Wv": np.asarray(inputs["Wv"], dtype=np.float32),
            "Wo": np.asarray(inputs["Wo"], dtype=np.float32),
            "bo": np.asarray(inputs["bo"], dtype=np.float32),
        }
        in_maps.append(m)
    res = run_bass_kernel_spmd(nc, in_maps, list(range(B)), trace=trace, **kw)
    out = np.stack(
        [np.asarray(res.results[i]["y"], dtype=np.float32) for i in range(B)], axis=0
    )
    return out, res


def kernel(**inputs):
    out, _ = run(inputs)
    return out


# revision 20
# speedup vs baseline: 1.1680x; 1.0098x over previous
"""Cross-attention Trainium2 kernel (8-core data-parallel over batch).

Per-core computation (one batch element per NeuronCore):
  q = x @ Wq; k = ctx @ Wk; v = ctx @ Wv
  attn = softmax((q k^T) / sqrt(dh)); out = attn @ v; y = out @ Wo + bo

v2 structure (vs the v1 baseline):
  - weights arrive fp32 over the two HWDGE rings (sync/scalar) and are cast
    to bf16 on ACT/DVE, so the serial SWDGE (gpsimd cast-load) queue only
    carries the per-group x tiles -> the PE starts ~10us earlier.
  - xT via PE transposes + DVE copies (the DMA XBAR transpose path measured
    slower end-to-end: Tile serializes dma_start_transpose against all other
    in-flight DMAs as a HW-deadlock guard, which convoys the x/weight loads).
  - scores per head pair are emitted back-to-back as 64x128 row tiles
    (partition bases 0/64) into one 2-bank PSUM tile; the PE runs both
    concurrently (measured pair span ~320ns vs 432 serial), and a single
    ACT exp sweeps the 2-bank tile.
  - softmax denominators: col-tiled (128x64) pair matmuls with a shared
    [77->128, 64] all-ones bf16 stationary write sum_ctx(exp_h) broadcast
    across the 64 out partitions of each head; same tile mode as the
    attention-output pair matmuls (no extra PE mode switch).
  - attn-output pairs unchanged (col tiles 0/64), evicted with *1/rowsum.
  - y stores ride the sync HWDGE ring; everything on-chip stays bf16 for
    matmul operands with fp32 PSUM accumulation.
"""

import numpy as np

import concourse.bass as bass
import concourse.tile as tile
from concourse import bacc, mybir
from concourse.bass_utils import run_bass_kernel_spmd
from concourse.masks import make_identity

F32 = mybir.dt.float32
BF16 = mybir.dt.bfloat16

B, N, M = 8, 4096, 77
QD, CD, H, DH = 512, 768, 8, 64
INNER = H * DH  # 512
P = 128
S = 512  # token group size
NQC = QD // P  # 4 qd chunks
NCC = CD // P  # 6 cd chunks
NIC = INNER // P  # 4 inner chunks
NTS = S // P  # 4 token sub-tiles per group
SCALE = DH ** -0.5
MP = 128  # context length padded to full partition width (zeros are inert)


def build_kernel(groups: int = N // S):
    nc = bacc.Bacc(None, target_bir_lowering=False, debug=False)

    x_d = nc.dram_tensor("x", [N, QD], F32, kind="ExternalInput")
    ctx_d = nc.dram_tensor("context", [M, CD], F32, kind="ExternalInput")
    wq_d = nc.dram_tensor("Wq", [QD, INNER], F32, kind="ExternalInput")
    wk_d = nc.dram_tensor("Wk", [CD, INNER], F32, kind="ExternalInput")
    wv_d = nc.dram_tensor("Wv", [CD, INNER], F32, kind="ExternalInput")
    wo_d = nc.dram_tensor("Wo", [INNER, QD], F32, kind="ExternalInput")
    bo_d = nc.dram_tensor("bo", [QD], F32, kind="ExternalInput")
    y_d = nc.dram_tensor("y", [N, QD], BF16, kind="ExternalOutput")

    from contextlib import ExitStack

    with tile.TileContext(nc) as tc, ExitStack() as st:
        consts = st.enter_context(tc.tile_pool(name="consts", bufs=1))
        kvp = st.enter_context(tc.tile_pool(name="kv", bufs=1))
        xin = st.enter_context(tc.tile_pool(name="xin", bufs=3))
        xtp = st.enter_context(tc.tile_pool(name="xt", bufs=2))
        qtp = st.enter_context(tc.tile_pool(name="qt", bufs=2))
        expp = st.enter_context(tc.tile_pool(name="expp", bufs=3))
        rcp = st.enter_context(tc.tile_pool(name="rcp", bufs=2))
        outp = st.enter_context(tc.tile_pool(name="outp", bufs=3))
        yp = st.enter_context(tc.tile_pool(name="yp", bufs=2))

        # PSUM: 8 banks. qf 2x[128,512] (2) + s 2x[128,2,512] (4) + r 2x[128,512] (2)
        ps_qf = st.enter_context(tc.tile_pool(name="ps_qf", bufs=2, space="PSUM"))
        ps_s = st.enter_context(tc.tile_pool(name="ps_s", bufs=2, space="PSUM"))
        ps_r = st.enter_context(tc.tile_pool(name="ps_r", bufs=2, space="PSUM"))

        # ---- tiny constants (engines only, no DMA) ------------------------------
        identity = consts.tile([P, P], BF16)
        make_identity(nc, identity)

        # all-ones [77, 64] stationary (zero-padded to 128 rows) for the
        # col-tiled rowsum matmuls
        ones64 = consts.tile([MP, DH], BF16)
        nc.vector.memset(ones64, 0.0)
        nc.vector.memset(ones64[:M, :], 1.0)

        # ---- loads: one serial SWDGE (gpsimd cast-load) queue, in need order ----
        # A single queue keeps DMA completion order == consumption order; the
        # HWDGE-parallel variant measured worse (SDMA round-robins the rings,
        # so the early-critical small loads finish last).
        def load_x(g):
            x_g = xin.tile([P, NTS, QD], BF16)
            nc.gpsimd.dma_start(
                out=x_g,
                in_=x_d[g * S : (g + 1) * S, :].rearrange("(t p) q -> p t q", p=P),
            )
            return x_g

        ctx_sb = kvp.tile([MP, CD], BF16)
        nc.vector.memset(ctx_sb, 0.0)
        nc.gpsimd.dma_start(out=ctx_sb[:M, :], in_=ctx_d[:, :])

        x_pre = [load_x(0)]

        wk_sb = consts.tile([P, NCC, INNER], BF16)
        nc.gpsimd.dma_start(
            out=wk_sb, in_=wk_d.ap().rearrange("(c p) n -> p c n", p=P)
        )

        wq_sb = consts.tile([P, NQC, INNER], BF16)
        nc.gpsimd.dma_start(
            out=wq_sb, in_=wq_d.ap().rearrange("(c p) n -> p c n", p=P)
        )

        x_pre.append(load_x(1))

        wv_sb = consts.tile([P, NCC, INNER], BF16)
        nc.gpsimd.dma_start(
            out=wv_sb, in_=wv_d.ap().rearrange("(c p) n -> p c n", p=P)
        )

        wo_sb = consts.tile([P, NIC, QD], BF16)
        nc.gpsimd.dma_start(
            out=wo_sb, in_=wo_d.ap().rearrange("(c p) n -> p c n", p=P)
        )

        bo_bc = consts.tile([P, QD], F32)
        bo_ap = bo_d.ap()
        nc.gpsimd.dma_start(
            out=bo_bc, in_=bass.AP(bo_ap.tensor, bo_ap.offset, [[0, P], [1, QD]])
        )

        # PE warmup: dummy transposes while the first DMAs are in flight keep
        # the HAM clock gate at 8/8 so the real prologue matmuls run at 2.4GHz
        for _ in range(24):
            ptw = ps_r.tile([P, P], BF16, tag="ps_r")
            nc.tensor.transpose(ptw, identity, identity)

        # ---- context projections (tiny) -----------------------------------------
        ctxT = kvp.tile([P, NCC, MP], BF16)
        for cc in range(NCC):
            pt = ps_r.tile([P, MP], BF16, tag="ps_r")
            nc.tensor.transpose(pt, ctx_sb[:, cc * P : (cc + 1) * P], identity)
            nc.vector.tensor_copy(out=ctxT[:, cc, :], in_=pt)

        kT = kvp.tile([P, NIC, MP], BF16)
        for ic in range(NIC):
            pk = ps_qf.tile([P, MP], F32, tag="ps_qf")
            for cc in range(NCC):
                nc.tensor.matmul(
                    pk,
                    wk_sb[:, cc, ic * P : (ic + 1) * P],
                    ctxT[:, cc, :],
                    start=(cc == 0),
                    stop=(cc == NCC - 1),
                )
            nc.vector.tensor_copy(out=kT[:, ic, :], in_=pk)

        # v projection is deferred until after front(0): Wv lands late in the
        # serial SWDGE queue, and emitting it here would park its matmuls at
        # the head of the in-order PE queue, stalling group 0/1 work behind it.
        v_sb = kvp.tile([MP, INNER], BF16)

        def emit_vproj():
            pv = ps_s.tile([MP, INNER], F32, tag="ps_s")
            for cc in range(NCC):
                nc.tensor.matmul(
                    pv,
                    ctxT[:, cc, :],
                    wv_sb[:, cc, :],
                    start=(cc == 0),
                    stop=(cc == NCC - 1),
                )
            nc.vector.tensor_copy(out=v_sb, in_=pv)

        # ---- main loop over token groups ----------------------------------------
        # Software-pipelined emission: group g's rowsums / attention-output /
        # final projection are emitted one iteration later, after group g+1's
        # front block, so the scheduler can fill PE stalls (exp latency)
        # with back-phase matmuls.

        def emit_front(g):
            x_g = x_pre[g]
            if g + 2 < groups:
                x_pre.append(load_x(g + 2))

            # transpose x tiles: xT[p, c, t*128+j] = x[t*128+..., c*128+p];
            # 4 PE transposes land in one psum bank, one DVE copy per chunk
            xT = xtp.tile([P, NQC, S], BF16)
            for c in range(NQC):
                pt = ps_r.tile([P, S], BF16, tag="ps_r")
                for ts in range(NTS):
                    nc.tensor.transpose(
                        pt[:, ts * P : (ts + 1) * P],
                        x_g[:, ts, c * P : (c + 1) * P],
                        identity,
                    )
                nc.vector.tensor_copy(out=xT[:, c, :], in_=pt)

            # qT[inner, tok]
            qT = qtp.tile([P, NIC, S], BF16)
            for ic in range(NIC):
                pq = ps_qf.tile([P, S], F32, tag="ps_qf")
                for c in range(NQC):
                    nc.tensor.matmul(
                        pq,
                        wq_sb[:, c, ic * P : (ic + 1) * P],
                        xT[:, c, :],
                        start=(c == 0),
                        stop=(c == NQC - 1),
                    )
                nc.scalar.copy(out=qT[:, ic, :], in_=pq)

            # scores -> exp per head pair: the pair runs as concurrent 64x128
            # row tiles (partition bases 0/64) into one 2-bank psum tile, then
            # one ACT exp sweeps both banks.
            exp_g = expp.tile([MP, H, S], BF16)
            for pp in range(H // 2):
                sp = ps_s.tile([MP, 2, S], F32, tag="ps_s")
                nc.tensor.matmul(
                    sp[:, 0, :],
                    kT[0:DH, pp, :],
                    qT[0:DH, pp, :],
                    start=True,
                    stop=True,
                )
                nc.tensor.matmul(
                    sp[:, 1, :],
                    kT[DH : 2 * DH, pp, :],
                    qT[DH : 2 * DH, pp, :],
                    start=True,
                    stop=True,
                )
                nc.scalar.activation(
                    out=exp_g[:, 2 * pp : 2 * pp + 2, :],
                    in_=sp,
                    func=mybir.ActivationFunctionType.Exp,
                    scale=SCALE,
                )
            return exp_g

        def emit_attn(g, exp_g):
            # rowsums, broadcast across each head's 64 out partitions by the
            # col-tiled ones matmuls; then reciprocal per pair
            rec_g = rcp.tile([P, H // 2, S], F32)
            for pp in range(H // 2):
                pr = ps_r.tile([P, S], F32, tag="ps_r")
                nc.tensor.matmul(
                    pr[0:DH, :],
                    ones64,
                    exp_g[:, 2 * pp, :],
                    start=True,
                    stop=True,
                    tile_position=(0, 0),
                )
                nc.tensor.matmul(
                    pr[DH : 2 * DH, :],
                    ones64,
                    exp_g[:, 2 * pp + 1, :],
                    start=True,
                    stop=True,
                    tile_position=(0, DH),
                )
                nc.vector.reciprocal_approx_fast(out=rec_g[:, pp, :], in_=pr)

            # outT (unnormalized) * (1/r); pair-packed into one bank
            outT = outp.tile([P, NIC, S], BF16)
            for pp in range(H // 2):
                po = ps_r.tile([P, S], F32, tag="ps_r")
                for side in range(2):
                    h = 2 * pp + side
                    nc.tensor.matmul(
                        po[side * DH : (side + 1) * DH, :],
                        v_sb[:, h * DH : (h + 1) * DH],
                        exp_g[:, h, :],
                        start=True,
                        stop=True,
                        tile_position=(0, side * DH),
                    )
                nc.vector.tensor_mul(
                    out=outT[:, pp, :], in0=po, in1=rec_g[:, pp, :]
                )
            return outT

        def emit_proj(g, outT):
            # final projection + bias; the last group streams each token
            # sub-tile out as soon as its bias-add lands (shorter tail)
            tok = slice(g * S, (g + 1) * S)
            y_g = yp.tile([P, NTS, QD], BF16)
            y_view = y_d[tok, :].rearrange("(t p) q -> p t q", p=P)
            last = g == groups - 1
            for ts in range(NTS):
                pf = ps_qf.tile([P, QD], F32, tag="ps_qf")
                for ic in range(NIC):
                    nc.tensor.matmul(
                        pf,
                        outT[:, ic, ts * P : (ts + 1) * P],
                        wo_sb[:, ic, :],
                        start=(ic == 0),
                        stop=(ic == NIC - 1),
                    )
                nc.vector.tensor_add(out=y_g[:, ts, :], in0=pf, in1=bo_bc)
                if last:
                    nc.sync.dma_start(out=y_view[:, ts, :], in_=y_g[:, ts, :])
            if not last:
                nc.sync.dma_start(out=y_view, in_=y_g)

        # pipeline: front(g) | attn(g-1) | proj(g-2)
        attn_q = []  # (g, exp_g) awaiting attn
        proj_q = []  # (g, outT) awaiting final projection
        for g in range(groups):
            exp_g = emit_front(g)
            if g == 0:
                emit_vproj()
            if attn_q:
                ga, ea = attn_q.pop(0)
                proj_q.append((ga, emit_attn(ga, ea)))
            if len(proj_q) > 1 or g == groups - 1:
                gp, op_ = proj_q.pop(0)
                emit_proj(gp, op_)
            attn_q.append((g, exp_g))
        while attn_q:
            ga, ea = attn_q.pop(0)
            proj_q.append((ga, emit_attn(ga, ea)))
        while proj_q:
            gp, op_ = proj_q.pop(0)
            emit_proj(gp, op_)

    nc.compile()
    return nc


_CACHE = {}


def _get_nc():
    if "nc" not in _CACHE:
        _CACHE["nc"] = build_kernel()
    return _CACHE["nc"]


def run(inputs, trace=False, **kw):
    nc = _get_nc()
    in_maps = []
    for i in range(B):
        m = {
            "x": np.asarray(inputs["x"][i], dtype=np.float32),
            "context": np.asarray(inputs["context"][i], dtype=np.float32),
            "Wq": np.asarray(inputs["Wq"], dtype=np.float32),
            "Wk": np.asarray(inputs["Wk"], dtype=np.float32),
            "Wv": np.asarray(inputs["Wv"], dtype=np.float32),
            "Wo": np.asarray(inputs["Wo"], dtype=np.float32),
            "bo": np.asarray(inputs["bo"], dtype=np.float32),
        }
        in_maps.append(m)
    res = run_bass_kernel_spmd(nc, in_maps, list(range(B)), trace=trace, **kw)
    out = np.stack(
        [np.asarray(res.results[i]["y"], dtype=np.float32) for i in range(B)], axis=0
    )
    return out, res


def kernel(**inputs):
    out, _ = run(inputs)
    return out


# revision 21
# speedup vs baseline: 1.1710x; 1.0026x over previous
"""Cross-attention Trainium2 kernel (8-core data-parallel over batch).

Per-core computation (one batch element per NeuronCore):
  q = x @ Wq; k = ctx @ Wk; v = ctx @ Wv
  attn = softmax((q k^T) / sqrt(dh)); out = attn @ v; y = out @ Wo + bo

v2 structure (vs the v1 baseline):
  - weights arrive fp32 over the two HWDGE rings (sync/scalar) and are cast
    to bf16 on ACT/DVE, so the serial SWDGE (gpsimd cast-load) queue only
    carries the per-group x tiles -> the PE starts ~10us earlier.
  - xT via PE transposes + DVE copies (the DMA XBAR transpose path measured
    slower end-to-end: Tile serializes dma_start_transpose against all other
    in-flight DMAs as a HW-deadlock guard, which convoys the x/weight loads).
  - scores per head pair are emitted back-to-back as 64x128 row tiles
    (partition bases 0/64) into one 2-bank PSUM tile; the PE runs both
    concurrently (measured pair span ~320ns vs 432 serial), and a single
    ACT exp sweeps the 2-bank tile.
  - softmax denominators: col-tiled (128x64) pair matmuls with a shared
    [77->128, 64] all-ones bf16 stationary write sum_ctx(exp_h) broadcast
    across the 64 out partitions of each head; same tile mode as the
    attention-output pair matmuls (no extra PE mode switch).
  - attn-output pairs unchanged (col tiles 0/64), evicted with *1/rowsum.
  - y stores ride the sync HWDGE ring; everything on-chip stays bf16 for
    matmul operands with fp32 PSUM accumulation.
"""

import numpy as np

import concourse.bass as bass
import concourse.tile as tile
from concourse import bacc, mybir
from concourse.bass_utils import run_bass_kernel_spmd
from concourse.masks import make_identity

F32 = mybir.dt.float32
BF16 = mybir.dt.bfloat16

B, N, M = 8, 4096, 77
QD, CD, H, DH = 512, 768, 8, 64
INNER = H * DH  # 512
P = 128
S = 512  # token group size
NQC = QD // P  # 4 qd chunks
NCC = CD // P  # 6 cd chunks
NIC = INNER // P  # 4 inner chunks
NTS = S // P  # 4 token sub-tiles per group
SCALE = DH ** -0.5
MP = 128  # context length padded to full partition width (zeros are inert)


def build_kernel(groups: int = N // S):
    nc = bacc.Bacc(None, target_bir_lowering=False, debug=False)

    x_d = nc.dram_tensor("x", [N, QD], F32, kind="ExternalInput")
    ctx_d = nc.dram_tensor("context", [M, CD], F32, kind="ExternalInput")
    wq_d = nc.dram_tensor("Wq", [QD, INNER], F32, kind="ExternalInput")
    wk_d = nc.dram_tensor("Wk", [CD, INNER], F32, kind="ExternalInput")
    wv_d = nc.dram_tensor("Wv", [CD, INNER], F32, kind="ExternalInput")
    wo_d = nc.dram_tensor("Wo", [INNER, QD], F32, kind="ExternalInput")
    bo_d = nc.dram_tensor("bo", [QD], F32, kind="ExternalInput")
    y_d = nc.dram_tensor("y", [N, QD], BF16, kind="ExternalOutput")

    from contextlib import ExitStack

    with tile.TileContext(nc) as tc, ExitStack() as st:
        consts = st.enter_context(tc.tile_pool(name="consts", bufs=1))
        kvp = st.enter_context(tc.tile_pool(name="kv", bufs=1))
        xin = st.enter_context(tc.tile_pool(name="xin", bufs=3))
        xtp = st.enter_context(tc.tile_pool(name="xt", bufs=2))
        qtp = st.enter_context(tc.tile_pool(name="qt", bufs=2))
        expp = st.enter_context(tc.tile_pool(name="expp", bufs=3))
        rcp = st.enter_context(tc.tile_pool(name="rcp", bufs=2))
        outp = st.enter_context(tc.tile_pool(name="outp", bufs=3))
        yp = st.enter_context(tc.tile_pool(name="yp", bufs=3))

        # PSUM: 8 banks. qf 2x[128,512] (2) + s 2x[128,2,512] (4) + r 2x[128,512] (2)
        ps_qf = st.enter_context(tc.tile_pool(name="ps_qf", bufs=2, space="PSUM"))
        ps_s = st.enter_context(tc.tile_pool(name="ps_s", bufs=2, space="PSUM"))
        ps_r = st.enter_context(tc.tile_pool(name="ps_r", bufs=2, space="PSUM"))

        # ---- tiny constants (engines only, no DMA) ------------------------------
        identity = consts.tile([P, P], BF16)
        make_identity(nc, identity)

        # all-ones [77, 64] stationary (zero-padded to 128 rows) for the
        # col-tiled rowsum matmuls
        ones64 = consts.tile([MP, DH], BF16)
        nc.vector.memset(ones64, 0.0)
        nc.vector.memset(ones64[:M, :], 1.0)

        # ---- loads: one serial SWDGE (gpsimd cast-load) queue, in need order ----
        # A single queue keeps DMA completion order == consumption order; the
        # HWDGE-parallel variant measured worse (SDMA round-robins the rings,
        # so the early-critical small loads finish last).
        def load_x(g):
            x_g = xin.tile([P, NTS, QD], BF16)
            nc.gpsimd.dma_start(
                out=x_g,
                in_=x_d[g * S : (g + 1) * S, :].rearrange("(t p) q -> p t q", p=P),
            )
            return x_g

        ctx_sb = kvp.tile([MP, CD], BF16)
        nc.vector.memset(ctx_sb, 0.0)
        nc.gpsimd.dma_start(out=ctx_sb[:M, :], in_=ctx_d[:, :])

        x_pre = [load_x(0)]

        wk_sb = consts.tile([P, NCC, INNER], BF16)
        nc.gpsimd.dma_start(
            out=wk_sb, in_=wk_d.ap().rearrange("(c p) n -> p c n", p=P)
        )

        wq_sb = consts.tile([P, NQC, INNER], BF16)
        nc.gpsimd.dma_start(
            out=wq_sb, in_=wq_d.ap().rearrange("(c p) n -> p c n", p=P)
        )

        x_pre.append(load_x(1))

        wv_sb = consts.tile([P, NCC, INNER], BF16)
        nc.gpsimd.dma_start(
            out=wv_sb, in_=wv_d.ap().rearrange("(c p) n -> p c n", p=P)
        )

        wo_sb = consts.tile([P, NIC, QD], BF16)
        nc.gpsimd.dma_start(
            out=wo_sb, in_=wo_d.ap().rearrange("(c p) n -> p c n", p=P)
        )

        bo_bc = consts.tile([P, QD], F32)
        bo_ap = bo_d.ap()
        nc.gpsimd.dma_start(
            out=bo_bc, in_=bass.AP(bo_ap.tensor, bo_ap.offset, [[0, P], [1, QD]])
        )

        # PE warmup: dummy transposes while the first DMAs are in flight keep
        # the HAM clock gate at 8/8 so the real prologue matmuls run at 2.4GHz
        for _ in range(24):
            ptw = ps_r.tile([P, P], BF16, tag="ps_r")
            nc.tensor.transpose(ptw, identity, identity)

        # ---- context projections (tiny) -----------------------------------------
        ctxT = kvp.tile([P, NCC, MP], BF16)
        for cc in range(NCC):
            pt = ps_r.tile([P, MP], BF16, tag="ps_r")
            nc.tensor.transpose(pt, ctx_sb[:, cc * P : (cc + 1) * P], identity)
            nc.vector.tensor_copy(out=ctxT[:, cc, :], in_=pt)

        kT = kvp.tile([P, NIC, MP], BF16)
        for ic in range(NIC):
            pk = ps_qf.tile([P, MP], F32, tag="ps_qf")
            for cc in range(NCC):
                nc.tensor.matmul(
                    pk,
                    wk_sb[:, cc, ic * P : (ic + 1) * P],
                    ctxT[:, cc, :],
                    start=(cc == 0),
                    stop=(cc == NCC - 1),
                )
            nc.vector.tensor_copy(out=kT[:, ic, :], in_=pk)

        # v projection is deferred until after front(0): Wv lands late in the
        # serial SWDGE queue, and emitting it here would park its matmuls at
        # the head of the in-order PE queue, stalling group 0/1 work behind it.
        v_sb = kvp.tile([MP, INNER], BF16)

        def emit_vproj():
            pv = ps_s.tile([MP, INNER], F32, tag="ps_s")
            for cc in range(NCC):
                nc.tensor.matmul(
                    pv,
                    ctxT[:, cc, :],
                    wv_sb[:, cc, :],
                    start=(cc == 0),
                    stop=(cc == NCC - 1),
                )
            nc.vector.tensor_copy(out=v_sb, in_=pv)

        # ---- main loop over token groups ----------------------------------------
        # Software-pipelined emission: group g's rowsums / attention-output /
        # final projection are emitted one iteration later, after group g+1's
        # front block, so the scheduler can fill PE stalls (exp latency)
        # with back-phase matmuls.

        def emit_front(g):
            x_g = x_pre[g]
            if g + 2 < groups:
                x_pre.append(load_x(g + 2))

            # transpose x tiles: xT[p, c, t*128+j] = x[t*128+..., c*128+p];
            # 4 PE transposes land in one psum bank, one DVE copy per chunk
            xT = xtp.tile([P, NQC, S], BF16)
            for c in range(NQC):
                pt = ps_r.tile([P, S], BF16, tag="ps_r")
                for ts in range(NTS):
                    nc.tensor.transpose(
                        pt[:, ts * P : (ts + 1) * P],
                        x_g[:, ts, c * P : (c + 1) * P],
                        identity,
                    )
                nc.vector.tensor_copy(out=xT[:, c, :], in_=pt)

            # qT[inner, tok]
            qT = qtp.tile([P, NIC, S], BF16)
            for ic in range(NIC):
                pq = ps_qf.tile([P, S], F32, tag="ps_qf")
                for c in range(NQC):
                    nc.tensor.matmul(
                        pq,
                        wq_sb[:, c, ic * P : (ic + 1) * P],
                        xT[:, c, :],
                        start=(c == 0),
                        stop=(c == NQC - 1),
                    )
                nc.scalar.copy(out=qT[:, ic, :], in_=pq)

            # scores -> exp per head pair: the pair runs as concurrent 64x128
            # row tiles (partition bases 0/64) into one 2-bank psum tile, then
            # one ACT exp sweeps both banks.
            exp_g = expp.tile([MP, H, S], BF16)
            for pp in range(H // 2):
                sp = ps_s.tile([MP, 2, S], F32, tag="ps_s")
                nc.tensor.matmul(
                    sp[:, 0, :],
                    kT[0:DH, pp, :],
                    qT[0:DH, pp, :],
                    start=True,
                    stop=True,
                )
                nc.tensor.matmul(
                    sp[:, 1, :],
                    kT[DH : 2 * DH, pp, :],
                    qT[DH : 2 * DH, pp, :],
                    start=True,
                    stop=True,
                )
                nc.scalar.activation(
                    out=exp_g[:, 2 * pp : 2 * pp + 2, :],
                    in_=sp,
                    func=mybir.ActivationFunctionType.Exp,
                    scale=SCALE,
                )
            return exp_g

        def emit_attn(g, exp_g):
            # rowsums, broadcast across each head's 64 out partitions by the
            # col-tiled ones matmuls; then reciprocal per pair
            rec_g = rcp.tile([P, H // 2, S], F32)
            for pp in range(H // 2):
                pr = ps_r.tile([P, S], F32, tag="ps_r")
                nc.tensor.matmul(
                    pr[0:DH, :],
                    ones64,
                    exp_g[:, 2 * pp, :],
                    start=True,
                    stop=True,
                    tile_position=(0, 0),
                )
                nc.tensor.matmul(
                    pr[DH : 2 * DH, :],
                    ones64,
                    exp_g[:, 2 * pp + 1, :],
                    start=True,
                    stop=True,
                    tile_position=(0, DH),
                )
                nc.vector.reciprocal_approx_fast(out=rec_g[:, pp, :], in_=pr)

            # outT (unnormalized) * (1/r); pair-packed into one bank
            outT = outp.tile([P, NIC, S], BF16)
            for pp in range(H // 2):
                po = ps_r.tile([P, S], F32, tag="ps_r")
                for side in range(2):
                    h = 2 * pp + side
                    nc.tensor.matmul(
                        po[side * DH : (side + 1) * DH, :],
                        v_sb[:, h * DH : (h + 1) * DH],
                        exp_g[:, h, :],
                        start=True,
                        stop=True,
                        tile_position=(0, side * DH),
                    )
                nc.vector.tensor_mul(
                    out=outT[:, pp, :], in0=po, in1=rec_g[:, pp, :]
                )
            return outT

        def emit_proj(g, outT):
            # final projection + bias; the last group streams each token
            # sub-tile out as soon as its bias-add lands (shorter tail)
            tok = slice(g * S, (g + 1) * S)
            y_g = yp.tile([P, NTS, QD], BF16)
            y_view = y_d[tok, :].rearrange("(t p) q -> p t q", p=P)
            last = g == groups - 1
            for ts in range(NTS):
                pf = ps_qf.tile([P, QD], F32, tag="ps_qf")
                for ic in range(NIC):
                    nc.tensor.matmul(
                        pf,
                        outT[:, ic, ts * P : (ts + 1) * P],
                        wo_sb[:, ic, :],
                        start=(ic == 0),
                        stop=(ic == NIC - 1),
                    )
                nc.vector.tensor_add(out=y_g[:, ts, :], in0=pf, in1=bo_bc)
                if last:
                    nc.sync.dma_start(out=y_view[:, ts, :], in_=y_g[:, ts, :])
            if not last:
                nc.sync.dma_start(out=y_view, in_=y_g)

        # pipeline: front(g) | attn(g-1) | proj(g-2)
        attn_q = []  # (g, exp_g) awaiting attn
        proj_q = []  # (g, outT) awaiting final projection
        for g in range(groups):
            exp_g = emit_front(g)
            if g == 0:
                emit_vproj()
            if attn_q:
                ga, ea = attn_q.pop(0)
                proj_q.append((ga, emit_attn(ga, ea)))
            if len(proj_q) > 1 or g == groups - 1:
                gp, op_ = proj_q.pop(0)
                emit_proj(gp, op_)
            attn_q.append((g, exp_g))
        while attn_q:
            ga, ea = attn_q.pop(0)
            proj_q.append((ga, emit_attn(ga, ea)))
        while proj_q:
            gp, op_ = proj_q.pop(0)
            emit_proj(gp, op_)

    nc.compile()
    return nc


_CACHE = {}


def _get_nc():
    if "nc" not in _CACHE:
        _CACHE["nc"] = build_kernel()
    return _CACHE["nc"]


def run(inputs, trace=False, **kw):
    nc = _get_nc()
    in_maps = []
    for i in range(B):
        m = {
            "x": np.asarray(inputs["x"][i], dtype=np.float32),
            "context": np.asarray(inputs["context"][i], dtype=np.float32),
            "Wq": np.asarray(inputs["Wq"], dtype=np.float32),
            "Wk": np.asarray(inputs["Wk"], dtype=np.float32),
            "Wv": np.asarray(inputs["Wv"], dtype=np.float32),
            "Wo": np.asarray(inputs["Wo"], dtype=np.float32),
            "bo": np.asarray(inputs["bo"], dtype=np.float32),
        }
        in_maps.append(m)
    res = run_bass_kernel_spmd(nc, in_maps, list(range(B)), trace=trace, **kw)
    out = np.stack(
        [np.asarray(res.results[i]["y"], dtype=np.float32) for i in range(B)], axis=0
    )
    return out, res


def kernel(**inputs):
    out, _ = run(inputs)
    return out
